# revision 1
# baseline (speedup 1.0000x reference)
"""Causal self-attention on 8 trn2 NeuronCores.

Sharding: core c handles batch b = c//4 and heads 4*(c%4) .. 4*(c%4)+3
(data parallel on B, tensor parallel on the 16 heads). Each core computes
its 4 heads' attention plus the corresponding slice of the output
projection; the host sums the 4 partial projections per batch and adds bo.

On-chip layout is feature-major ("transposed"): qT/kT are [head_dim, seq],
scores are computed as sT[k, q] so the attention@v matmul needs no
transposes. The softmax denominator comes from an extra all-ones column
appended to Wv (so ctx PSUM row 64 accumulates sum_k exp). Normalization
broadcasts 1/denom across partitions via a DRAM bounce.

All matmuls run in float32r (TF32-like fp32 mode, 4x faster than fp32,
measured ~1.5e-4 L2 error per matmul).
"""

import sys

sys.path.insert(0, "/opt/trn_rl_repo")

import numpy as np

import concourse.bass as bass
import concourse.tile as tile
from concourse import bacc, mybir
from concourse.bass_utils import run_bass_kernel_spmd

B, S, D, H = 2, 2048, 1024, 16
HD = D // H            # 64
NCORES = 8
HPC = 4                # heads per core
DPC = HPC * HD         # 256 feature dims per core
QT = 512               # q tile (free dim of score matmuls)
KC = 128               # k chunk (partition dim of transposed scores)
NQT = S // QT          # 4
NKC = S // KC          # 16
VW = HPC * (HD + 1)    # 260: v with ones column per head

F32 = mybir.dt.float32
F32R = mybir.dt.float32r
import os
import ml_dtypes
USE_BF16 = os.environ.get("KBF16", "0") == "1"
DT = mybir.dt.bfloat16 if USE_BF16 else F32R
NPDT = ml_dtypes.bfloat16 if USE_BF16 else np.float32

_cache = {}


def _build(blocks, n_pat):
    """blocks: per q-tile, tuple of (ki, pat_idx|None) chunks to compute."""
    nc = bacc.Bacc(
        "TRN2",
        target_bir_lowering=False,
        debug=False,
        enable_asserts=False,
        num_devices=NCORES,
    )

    xt_d = nc.dram_tensor("xt", [D, S], DT, kind="ExternalInput").ap()
    wq_d = nc.dram_tensor("wq", [D, DPC], DT, kind="ExternalInput").ap()
    wk_d = nc.dram_tensor("wk", [D, DPC], DT, kind="ExternalInput").ap()
    wv_d = nc.dram_tensor("wv", [D, VW], DT, kind="ExternalInput").ap()
    wo_d = nc.dram_tensor("wo", [DPC, D], DT, kind="ExternalInput").ap()
    bq_d = nc.dram_tensor("bq", [128, 2], F32, kind="ExternalInput").ap()
    bk_d = nc.dram_tensor("bk", [128, 2], F32, kind="ExternalInput").ap()
    bv_d = nc.dram_tensor("bv", [VW], F32, kind="ExternalInput").ap()
    mp_d = nc.dram_tensor("mp", [128, max(n_pat, 1) * QT], DT, kind="ExternalInput").ap()
    id_d = nc.dram_tensor("ident", [128, 128], DT, kind="ExternalInput").ap()
    out_d = nc.dram_tensor("out", [S, D], F32, kind="ExternalOutput").ap()

    with tile.TileContext(nc) as tc:
        with (
            tc.tile_pool(name="consts", bufs=1) as consts,
            tc.tile_pool(name="mm_ps", bufs=2, space="PSUM") as mm_ps,
            tc.tile_pool(name="st_ps", bufs=2, space="PSUM") as st_ps,
            tc.tile_pool(name="ctx_ps", bufs=2, space="PSUM") as ctx_ps,
            tc.tile_pool(name="op_ps", bufs=2, space="PSUM") as op_ps,
            tc.tile_pool(name="work", bufs=4) as work,
            tc.tile_pool(name="norm", bufs=2) as norm,
            tc.tile_pool(name="ctxn", bufs=2) as ctxn,
            tc.tile_pool(name="stage", bufs=3) as stage,
        ):
            # ---- resident loads (ordered so PE can start after wq + x block 0) ----
            wq_sb = consts.tile([128, 8, DPC], DT)
            for kc in range(8):
                nc.sync.dma_start(out=wq_sb[:, kc, :], in_=wq_d[kc * 128:(kc + 1) * 128, :])
            bq_sb = consts.tile([128, 2], F32)
            bk_sb = consts.tile([128, 2], F32)
            nc.sync.dma_start(out=bq_sb, in_=bq_d)
            nc.sync.dma_start(out=bk_sb, in_=bk_d)
            # x^T split into 4 sequence blocks of 512 so compute starts early
            xtb = []
            for nb in range(NQT):
                xtb_t = consts.tile([128, 8, QT], DT, tag=f"xtb{nb}")
                xtb.append(xtb_t)
            for kc in range(8):
                nc.sync.dma_start(
                    out=xtb[0][:, kc, :], in_=xt_d[kc * 128:(kc + 1) * 128, 0:QT]
                )
            wk_sb = consts.tile([128, 8, DPC], DT)
            wv_sb = consts.tile([128, 8, VW], DT)
            for kc in range(8):
                nc.sync.dma_start(out=wk_sb[:, kc, :], in_=wk_d[kc * 128:(kc + 1) * 128, :])
            for kc in range(8):
                nc.sync.dma_start(out=wv_sb[:, kc, :], in_=wv_d[kc * 128:(kc + 1) * 128, :])
            bv_sb = consts.tile([128, VW], F32)
            nc.sync.dma_start(
                out=bv_sb,
                in_=bass.AP(tensor=bv_d.tensor, offset=0, ap=[[0, 128], [1, VW]]),
            )
            for nb in range(1, NQT):
                for kc in range(8):
                    nc.sync.dma_start(
                        out=xtb[nb][:, kc, :],
                        in_=xt_d[kc * 128:(kc + 1) * 128, nb * QT:(nb + 1) * QT],
                    )
            mp_sb = consts.tile([128, max(n_pat, 1), QT], DT)
            for p in range(max(n_pat, 1)):
                nc.sync.dma_start(out=mp_sb[:, p, :], in_=mp_d[:, p * QT:(p + 1) * QT])
            id_sb = consts.tile([128, 128], DT)
            nc.sync.dma_start(out=id_sb, in_=id_d)
            wo_sb = consts.tile([128, 2, D], DT)
            for cc in range(2):
                nc.sync.dma_start(out=wo_sb[:, cc, :], in_=wo_d[cc * 128:(cc + 1) * 128, :])

            ones_f = consts.tile([65, HD], F32)
            nc.vector.memset(ones_f, 1.0)
            ones_r = consts.tile([65, HD], F32R)
            nc.vector.tensor_copy(ones_r, ones_f)

            # ---- phase A: qT/kT = W @ xT, v = x @ Wv_aug (feature-major q/k) ----
            qt_sb = consts.tile([128, 2, S], DT)
            kt_sb = consts.tile([128, 2, S], DT)
            v_sb = consts.tile([128, NKC, VW], DT)

            def phase_a_block(n):
                for m in range(2):
                    ps = mm_ps.tile([128, QT], F32, tag="mm")
                    for kc in range(8):
                        nc.tensor.matmul(
                            ps,
                            wq_sb[:, kc, m * 128:(m + 1) * 128],
                            xtb[n][:, kc, :],
                            start=(kc == 0), stop=(kc == 7),
                        )
                    nc.vector.tensor_scalar_add(
                        qt_sb[:, m, n * QT:(n + 1) * QT], ps, bq_sb[:, m:m + 1]
                    )
                    ps = mm_ps.tile([128, QT], F32, tag="mm")
                    for kc in range(8):
                        nc.tensor.matmul(
                            ps,
                            wk_sb[:, kc, m * 128:(m + 1) * 128],
                            xtb[n][:, kc, :],
                            start=(kc == 0), stop=(kc == 7),
                        )
                    nc.vector.tensor_scalar_add(
                        kt_sb[:, m, n * QT:(n + 1) * QT], ps, bk_sb[:, m:m + 1]
                    )
                for sc in range(4 * n, 4 * n + 4):
                    ps = mm_ps.tile([128, VW], F32, tag="mm")
                    for kc in range(8):
                        nc.tensor.matmul(
                            ps,
                            xtb[n][:, kc, (sc % 4) * 128:(sc % 4 + 1) * 128],
                            wv_sb[:, kc, :],
                            start=(kc == 0), stop=(kc == 7),
                        )
                    nc.vector.tensor_add(v_sb[:, sc, :], ps, bv_sb)

            # ---- phase B/C: attention + output projection per q tile ----
            def attention_qtile(qi):
                qsl = slice(qi * QT, (qi + 1) * QT)
                cn0 = ctxn.tile([128, QT], DT, tag="cn0")
                cn1 = ctxn.tile([128, QT], DT, tag="cn1")
                cn = [cn0, cn1]

                for h in (2, 3, 0, 1):
                    even = (h % 2 == 0)
                    mc = h // 2                    # feature chunk of this head
                    fo = (h % 2) * HD              # feature offset within chunk
                    chunks = blocks[qi]
                    ctx = ctx_ps.tile([HD + 1, QT], F32)
                    for i, (ki, pat) in enumerate(chunks):
                        st = st_ps.tile([128, QT], F32)
                        nc.tensor.matmul(
                            st,
                            kt_sb[fo:fo + HD, mc, ki * 128:(ki + 1) * 128],
                            qt_sb[fo:fo + HD, mc, qsl],
                            start=True, stop=(pat is None),
                        )
                        if pat is not None:
                            # add -2000 to masked entries (exp -> ~0) on the PE
                            nc.tensor.matmul(
                                st, id_sb, mp_sb[:, pat, :],
                                start=False, stop=True,
                            )
                        ex = work.tile([128, QT], DT)
                        nc.scalar.activation(
                            out=ex, in_=st,
                            func=mybir.ActivationFunctionType.Exp, scale=0.125,
                        )
                        nc.tensor.matmul(
                            ctx,
                            v_sb[:, ki, h * (HD + 1):(h + 1) * (HD + 1)],
                            ex,
                            start=(i == 0), stop=(i == len(chunks) - 1),
                        )
                    # Normalize: broadcast the denominator across the 64 ctx
                    # partitions with a K=1 matmul (ones x denom), take the
                    # reciprocal with the fast Newton DVE op (which also
                    # evacuates the broadcast out of PSUM), then multiply the
                    # ctx rows (still in PSUM) by it on the way to SBUF.
                    dn_sb = norm.tile([HD + 1, QT], F32R, tag="dn")
                    with nc.allow_low_precision(reason="f32r operand for bcast matmul"):
                        nc.vector.tensor_copy(dn_sb[HD:HD + 1, :], ctx[HD:HD + 1, :])
                    bc = op_ps.tile([HD, QT], F32, tag="op")
                    nc.tensor.matmul(
                        bc, ones_r[HD:HD + 1, :], dn_sb[HD:HD + 1, :],
                        start=True, stop=True,
                    )
                    rc = norm.tile([HD, QT], F32, tag="rc")
                    nc.vector.reciprocal_approx_fast(out=rc, in_=bc)
                    if even:
                        nc.vector.tensor_mul(cn[mc][0:HD, :], ctx[0:HD, :], rc)
                    else:
                        tmp2 = norm.tile([HD, QT], DT, tag="tmp2")
                        nc.vector.tensor_mul(tmp2, ctx[0:HD, :], rc)
                        nc.sync.dma_start(out=cn[mc][HD:2 * HD, :], in_=tmp2)
                # output projection for this q tile
                for qc in range(4):
                    for ne in range(2):
                        ps = op_ps.tile([128, QT], F32, tag="op")
                        for cc in (1, 0):
                            nc.tensor.matmul(
                                ps,
                                cn[cc][:, qc * 128:(qc + 1) * 128],
                                wo_sb[:, cc, ne * QT:(ne + 1) * QT],
                                start=(cc == 1), stop=(cc == 0),
                            )
                        so = stage.tile([128, QT], F32)
                        nc.vector.tensor_copy(so, ps)
                        nc.sync.dma_start(
                            out=out_d[qi * QT + qc * 128: qi * QT + (qc + 1) * 128,
                                      ne * QT:(ne + 1) * QT],
                            in_=so,
                        )

            # interleave: emit each attention q-tile right after the phase-A
            # block that completes its inputs (block index = max ki // 4)
            ready_at = [max(ki for ki, _ in blocks[qi]) // 4 for qi in range(NQT)]
            for n in range(NQT):
                phase_a_block(n)
                for qi in range(NQT):
                    if ready_at[qi] == n:
                        attention_qtile(qi)

    nc.compile()
    return nc


def _block_structure(mask):
    """Classify [QT x KC] score blocks from the runtime mask (mask[q, k])."""
    allowed = ~np.isneginf(np.asarray(mask, dtype=np.float32))
    pats = []
    pat_idx = {}
    blocks = []
    for qi in range(NQT):
        row = []
        for ki in range(NKC):
            sub = allowed[qi * QT:(qi + 1) * QT, ki * KC:(ki + 1) * KC]
            if not sub.any():
                continue
            if sub.all():
                row.append((ki, None))
            else:
                pat = np.ascontiguousarray(
                    np.where(sub.T, 0.0, -2000.0).astype(np.float32)
                )  # [128, 512] additive mask
                key = pat.tobytes()
                if key not in pat_idx:
                    pat_idx[key] = len(pats)
                    pats.append(pat)
                row.append((ki, pat_idx[key]))
        blocks.append(tuple(row))
    return tuple(blocks), pats


def kernel(x, mask, Wq, bq, Wk, bk, Wv, bv, Wo, bo):
    x = np.asarray(x, dtype=np.float32)
    blocks, pats = _block_structure(mask)
    n_pat = len(pats)
    key = (blocks, n_pat, USE_BF16)
    if key not in _cache:
        _cache[key] = _build(blocks, n_pat)
    nc = _cache[key]

    if n_pat:
        mp = np.concatenate(pats, axis=1)          # [128, n_pat*QT]
    else:
        mp = np.zeros((128, QT), dtype=np.float32)

    xt = [np.ascontiguousarray(x[b].T).astype(NPDT) for b in range(B)]
    in_maps = []
    for c in range(NCORES):
        b, hg = c // HPC, c % HPC
        hs = slice(hg * DPC, (hg + 1) * DPC)
        wv_aug = np.zeros((D, VW), dtype=np.float32)
        bv_aug = np.zeros(VW, dtype=np.float32)
        for j in range(HPC):
            base = j * (HD + 1)
            rows = slice(hg * DPC + j * HD, hg * DPC + (j + 1) * HD)
            wv_aug[:, base:base + HD] = np.asarray(Wv)[rows, :].T
            bv_aug[base:base + HD] = np.asarray(bv)[rows]
            bv_aug[base + HD] = 1.0
        in_maps.append({
            "xt": xt[b],
            "wq": np.ascontiguousarray(np.asarray(Wq)[hs, :].T).astype(NPDT),
            "wk": np.ascontiguousarray(np.asarray(Wk)[hs, :].T).astype(NPDT),
            "wv": wv_aug.astype(NPDT),
            "wo": np.ascontiguousarray(np.asarray(Wo)[:, hs].T).astype(NPDT),
            "bq": np.ascontiguousarray(np.asarray(bq)[hs].reshape(2, 128).T),
            "bk": np.ascontiguousarray(np.asarray(bk)[hs].reshape(2, 128).T),
            "bv": bv_aug,
            "mp": mp.astype(NPDT),
            "ident": np.eye(128, dtype=np.float32).astype(NPDT),
        })

    res = run_bass_kernel_spmd(nc, in_maps, core_ids=list(range(NCORES))).results
    out = np.empty((B, S, D), dtype=np.float32)
    for b in range(B):
        acc = res[b * HPC]["out"].astype(np.float32).copy()
        for g in range(1, HPC):
            acc += res[b * HPC + g]["out"]
        out[b] = acc + np.asarray(bo, dtype=np.float32)[None, :]
    return out



# revision 3
# speedup vs baseline: 1.0147x; 1.0147x over previous
"""Causal self-attention on 8 trn2 NeuronCores.

Sharding: core c handles batch b = c//4 and heads 4*(c%4) .. 4*(c%4)+3
(data parallel on B, tensor parallel on the 16 heads). Each core computes
its 4 heads' attention plus the corresponding slice of the output
projection; the host sums the 4 partial projections per batch and adds bo.

On-chip layout is feature-major ("transposed"): qT/kT are [head_dim, seq],
scores are computed as sT[k, q] so the attention@v matmul needs no
transposes. The softmax denominator comes from an extra all-ones column
appended to Wv (so ctx PSUM row 64 accumulates sum_k exp). Normalization
broadcasts 1/denom across partitions via a tiny K=1 matmul.

Matmuls run in bfloat16 (hw-measured 2x faster than fp32/f32r for both
MATMUL and LDWEIGHTS on trn2). Causal masking is done with gpsimd
affine_select on the exp'd scores (keep iff q >= k, i.e. f - p - c >= 0
in the transposed block layout) instead of PE mask-add matmuls. Score
blocks are exp'd two at a time from a 2-bank PSUM tile to halve the
scalar-engine instruction count.
"""

import os
import sys

sys.path.insert(0, "/opt/trn_rl_repo")

import numpy as np
import ml_dtypes

import concourse.bass as bass
import concourse.tile as tile
from concourse import bacc, mybir
from concourse.bass_utils import run_bass_kernel_spmd

B, S, D, H = 2, 2048, 1024, 16
HD = D // H            # 64
NCORES = 8
HPC = 4                # heads per core
DPC = HPC * HD         # 256 feature dims per core
QT = 512               # q tile (free dim of score matmuls)
KC = 128               # k chunk (partition dim of transposed scores)
NQT = S // QT          # 4
NKC = S // KC          # 16
VW = HPC * (HD + 1)    # 260: v with ones column per head

F32 = mybir.dt.float32
F32R = mybir.dt.float32r
USE_BF16 = os.environ.get("KBF16", "1") == "1"
DT = mybir.dt.bfloat16 if USE_BF16 else F32R
NPDT = ml_dtypes.bfloat16 if USE_BF16 else np.float32

_cache = {}

# chunk kinds in the block structure
FULL = ("full",)


def _dram_ap(t, offset, dims):
    """dims: list of (elem_stride, n). Builds a raw AP on a dram tensor."""
    return bass.AP(tensor=t.tensor, offset=offset, ap=[list(d) for d in dims])


def _build(blocks, n_pat):
    """blocks: per q-tile, tuple of (ki, kind) chunks; kind is FULL,
    ('stair', c) with keep iff f - p - c >= 0, or ('mask', pat_idx)."""
    nc = bacc.Bacc(
        "TRN2",
        target_bir_lowering=False,
        debug=False,
        enable_asserts=False,
        num_devices=NCORES,
    )

    xt_d = nc.dram_tensor("xt", [D, S], DT, kind="ExternalInput").ap()
    wq_d = nc.dram_tensor("wq", [D, DPC], DT, kind="ExternalInput").ap()
    wk_d = nc.dram_tensor("wk", [D, DPC], DT, kind="ExternalInput").ap()
    wv_d = nc.dram_tensor("wv", [D, VW], DT, kind="ExternalInput").ap()
    wo_d = nc.dram_tensor("wo", [DPC, D], DT, kind="ExternalInput").ap()
    bq_d = nc.dram_tensor("bq", [128, 2], F32, kind="ExternalInput").ap()
    bk_d = nc.dram_tensor("bk", [128, 2], F32, kind="ExternalInput").ap()
    bv_d = nc.dram_tensor("bv", [VW], F32, kind="ExternalInput").ap()
    if n_pat:
        mp_d = nc.dram_tensor("mp", [128, n_pat * QT], DT, kind="ExternalInput").ap()
    out_d = nc.dram_tensor("out", [S, D], DT, kind="ExternalOutput").ap()

    with tile.TileContext(nc) as tc:
        with (
            tc.tile_pool(name="consts", bufs=1) as consts,
            tc.tile_pool(name="pe_ps", bufs=2, space="PSUM") as pe_ps,
            tc.tile_pool(name="st_ps", bufs=2, space="PSUM") as st_ps,
            tc.tile_pool(name="ctx_ps", bufs=2, space="PSUM") as ctx_ps,
            tc.tile_pool(name="work", bufs=4) as work,
            tc.tile_pool(name="norm", bufs=4) as norm,
            tc.tile_pool(name="ctxn", bufs=4) as ctxn,
            tc.tile_pool(name="stage", bufs=3) as stage,
        ):
            # ---- resident loads: one DMA descriptor per tensor, ordered so
            # the first q matmuls can start as early as possible ----
            wq_sb = consts.tile([128, 8, DPC], DT)
            nc.sync.dma_start(
                out=wq_sb,
                in_=_dram_ap(wq_d, 0, [(DPC, 128), (128 * DPC, 8), (1, DPC)]),
            )
            xtb = []
            for nb in range(NQT):
                xtb_t = consts.tile([128, 8, QT], DT, tag=f"xtb{nb}")
                xtb.append(xtb_t)
            nc.sync.dma_start(
                out=xtb[0],
                in_=_dram_ap(xt_d, 0, [(S, 128), (128 * S, 8), (1, QT)]),
            )
            wk_sb = consts.tile([128, 8, DPC], DT)
            wv_sb = consts.tile([128, 8, VW], DT)
            nc.sync.dma_start(
                out=wk_sb,
                in_=_dram_ap(wk_d, 0, [(DPC, 128), (128 * DPC, 8), (1, DPC)]),
            )
            nc.sync.dma_start(
                out=wv_sb,
                in_=_dram_ap(wv_d, 0, [(VW, 128), (128 * VW, 8), (1, VW)]),
            )
            bq_sb = consts.tile([128, 2], F32)
            bk_sb = consts.tile([128, 2], F32)
            bv_sb = consts.tile([128, VW], F32)
            nc.sync.dma_start(out=bq_sb, in_=bq_d)
            nc.sync.dma_start(out=bk_sb, in_=bk_d)
            nc.sync.dma_start(out=bv_sb, in_=_dram_ap(bv_d, 0, [(0, 128), (1, VW)]))
            for nb in range(1, NQT):
                nc.sync.dma_start(
                    out=xtb[nb],
                    in_=_dram_ap(xt_d, nb * QT, [(S, 128), (128 * S, 8), (1, QT)]),
                )
            wo_sb = consts.tile([128, 2, D], DT)
            nc.sync.dma_start(
                out=wo_sb, in_=_dram_ap(wo_d, 0, [(D, 128), (128 * D, 2), (1, D)])
            )
            if n_pat:
                mp_sb = consts.tile([128, n_pat, QT], DT)
                nc.sync.dma_start(
                    out=mp_sb,
                    in_=_dram_ap(
                        mp_d, 0, [(n_pat * QT, 128), (QT, n_pat), (1, QT)]
                    ),
                )

            ones_f = consts.tile([65, HD], F32)
            nc.vector.memset(ones_f, 1.0)
            ones_r = consts.tile([65, HD], F32R)
            nc.vector.tensor_copy(ones_r, ones_f)

            # ---- phase A: qT/kT = W @ xT, v = x @ Wv_aug (feature-major q/k) ----
            qt_sb = consts.tile([128, 2, S], DT)
            kt_sb = consts.tile([128, 2, S], DT)
            v_sb = consts.tile([128, NKC, VW], DT)

            def phase_a_block(n):
                for m in range(2):
                    ps = pe_ps.tile([128, QT], F32, tag="pe")
                    for kc in range(8):
                        nc.tensor.matmul(
                            ps,
                            wq_sb[:, kc, m * 128:(m + 1) * 128],
                            xtb[n][:, kc, :],
                            start=(kc == 0), stop=(kc == 7),
                        )
                    nc.vector.tensor_scalar_add(
                        qt_sb[:, m, n * QT:(n + 1) * QT], ps, bq_sb[:, m:m + 1]
                    )
                    ps = pe_ps.tile([128, QT], F32, tag="pe")
                    for kc in range(8):
                        nc.tensor.matmul(
                            ps,
                            wk_sb[:, kc, m * 128:(m + 1) * 128],
                            xtb[n][:, kc, :],
                            start=(kc == 0), stop=(kc == 7),
                        )
                    nc.vector.tensor_scalar_add(
                        kt_sb[:, m, n * QT:(n + 1) * QT], ps, bk_sb[:, m:m + 1]
                    )
                for sc in range(4 * n, 4 * n + 4):
                    ps = pe_ps.tile([128, VW], F32, tag="pe")
                    for kc in range(8):
                        nc.tensor.matmul(
                            ps,
                            xtb[n][:, kc, (sc % 4) * 128:(sc % 4 + 1) * 128],
                            wv_sb[:, kc, :],
                            start=(kc == 0), stop=(kc == 7),
                        )
                    nc.vector.tensor_add(v_sb[:, sc, :], ps, bv_sb)

            # ---- phase B/C: attention + output projection per q tile ----
            def attention_qtile(qi):
                qsl = slice(qi * QT, (qi + 1) * QT)
                cn0 = ctxn.tile([128, QT], DT, tag="cn0")
                cn1 = ctxn.tile([128, QT], DT, tag="cn1")
                cn = [cn0, cn1]
                chunks = blocks[qi]
                # group score blocks in pairs: one 2-bank PSUM tile + one exp
                groups = [chunks[i:i + 2] for i in range(0, len(chunks), 2)]

                for h in (2, 3, 0, 1):
                    even = (h % 2 == 0)
                    mc = h // 2                    # feature chunk of this head
                    fo = (h % 2) * HD              # feature offset within chunk
                    ctx = ctx_ps.tile([HD + 1, QT], F32)
                    nleft = len(chunks)
                    for g in groups:
                        st = st_ps.tile([128, len(g), QT], F32, tag="st")
                        for si, (ki, kind) in enumerate(g):
                            nc.tensor.matmul(
                                st[:, si, :],
                                kt_sb[fo:fo + HD, mc, ki * 128:(ki + 1) * 128],
                                qt_sb[fo:fo + HD, mc, qsl],
                                start=True, stop=True,
                            )
                        ex = work.tile([128, len(g), QT], DT)
                        nc.scalar.activation(
                            out=ex, in_=st,
                            func=mybir.ActivationFunctionType.Exp, scale=0.125,
                        )
                        kinds = [kind for _, kind in g]
                        if any(k[0] == "stair" for k in kinds) and all(
                            k[0] in ("stair", "full") for k in kinds
                        ):
                            # keep iff f - p - c >= 0 per half (c=-512: keep all)
                            cs = [
                                (k[1] if k[0] == "stair" else -QT) for k in kinds
                            ]
                            ex2 = work.tile([128, len(g), QT], DT, tag="ex2")
                            pat = (
                                [[cs[0] - cs[1], 2], [1, QT]]
                                if len(g) == 2 else [[1, QT]]
                            )
                            nc.gpsimd.affine_select(
                                out=ex2, in_=ex,
                                pattern=pat,
                                compare_op=mybir.AluOpType.is_ge,
                                fill=0.0, base=-cs[0], channel_multiplier=-1,
                            )
                            ex = ex2
                        else:
                            for si, k in enumerate(kinds):
                                if k[0] == "full":
                                    continue
                                ex2 = work.tile([128, QT], DT, tag="exm")
                                if k[0] == "stair":
                                    nc.gpsimd.affine_select(
                                        out=ex2, in_=ex[:, si, :],
                                        pattern=[[1, QT]],
                                        compare_op=mybir.AluOpType.is_ge,
                                        fill=0.0, base=-k[1],
                                        channel_multiplier=-1,
                                    )
                                else:
                                    nc.gpsimd.tensor_mul(
                                        ex2, ex[:, si, :], mp_sb[:, k[1], :]
                                    )
                                nc.gpsimd.tensor_copy(ex[:, si, :], ex2)
                        for si, (ki, kind) in enumerate(g):
                            nc.tensor.matmul(
                                ctx,
                                v_sb[:, ki, h * (HD + 1):(h + 1) * (HD + 1)],
                                ex[:, si, :],
                                start=(nleft == len(chunks)), stop=(nleft == 1),
                            )
                            nleft -= 1
                    # Normalize: broadcast the denominator across the 64 ctx
                    # partitions with a K=1 matmul (ones x denom), take the
                    # reciprocal with the fast Newton DVE op, then multiply the
                    # ctx rows (still in PSUM) by it on the way to SBUF.
                    dn_sb = norm.tile([HD + 1, QT], F32R, tag="dn")
                    with nc.allow_low_precision(reason="f32r operand for bcast matmul"):
                        nc.vector.tensor_copy(dn_sb[HD:HD + 1, :], ctx[HD:HD + 1, :])
                    bc = pe_ps.tile([HD, QT], F32, tag="pe")
                    nc.tensor.matmul(
                        bc, ones_r[HD:HD + 1, :], dn_sb[HD:HD + 1, :],
                        start=True, stop=True,
                    )
                    rc = norm.tile([HD, QT], F32, tag="rc")
                    nc.vector.reciprocal_approx_fast(out=rc, in_=bc)
                    if even:
                        nc.vector.tensor_mul(cn[mc][0:HD, :], ctx[0:HD, :], rc)
                    else:
                        tmp2 = norm.tile([HD, QT], DT, tag="tmp2")
                        nc.vector.tensor_mul(tmp2, ctx[0:HD, :], rc)
                        nc.sync.dma_start(out=cn[mc][HD:2 * HD, :], in_=tmp2)
                # output projection for this q tile
                for qc in range(4):
                    so = stage.tile([128, 2, QT], DT)
                    for ne in range(2):
                        ps = pe_ps.tile([128, QT], F32, tag="pe")
                        for cc in (1, 0):
                            nc.tensor.matmul(
                                ps,
                                cn[cc][:, qc * 128:(qc + 1) * 128],
                                wo_sb[:, cc, ne * QT:(ne + 1) * QT],
                                start=(cc == 1), stop=(cc == 0),
                            )
                        nc.vector.tensor_copy(so[:, ne, :], ps)
                    nc.sync.dma_start(
                        out=out_d[qi * QT + qc * 128: qi * QT + (qc + 1) * 128, :],
                        in_=so,
                    )

            # interleave: emit each attention q-tile right after the phase-A
            # block that completes its inputs (block index = max ki // 4)
            ready_at = [max(ki for ki, _ in blocks[qi]) // 4 for qi in range(NQT)]
            for n in range(NQT):
                phase_a_block(n)
                for qi in range(NQT):
                    if ready_at[qi] == n:
                        attention_qtile(qi)

    nc.compile()
    return nc


def _block_structure(mask):
    """Classify [QT x KC] score blocks from the runtime mask (mask[q, k])."""
    allowed = ~np.isneginf(np.asarray(mask, dtype=np.float32))
    pats = []
    pat_idx = {}
    blocks = []
    for qi in range(NQT):
        row = []
        for ki in range(NKC):
            sub = allowed[qi * QT:(qi + 1) * QT, ki * KC:(ki + 1) * KC]
            if not sub.any():
                continue
            if sub.all():
                row.append((ki, FULL))
                continue
            # staircase: keep (p, f) iff f >= p + c, in transposed [k, q] view
            subT = sub.T  # [128 k, 512 q]
            c = None
            for cand in range(-QT + 1, KC + QT):
                ref = (np.arange(QT)[None, :] >= np.arange(KC)[:, None] + cand)
                if np.array_equal(subT, ref):
                    c = cand
                    break
            if c is not None:
                row.append((ki, ("stair", c)))
            else:
                pat = np.ascontiguousarray(
                    np.where(subT, 1.0, 0.0).astype(np.float32)
                )  # [128, 512] multiplicative mask
                key = pat.tobytes()
                if key not in pat_idx:
                    pat_idx[key] = len(pats)
                    pats.append(pat)
                row.append((ki, ("mask", pat_idx[key])))
        blocks.append(tuple(row))
    return tuple(blocks), pats


def kernel(x, mask, Wq, bq, Wk, bk, Wv, bv, Wo, bo):
    x = np.asarray(x, dtype=np.float32)
    blocks, pats = _block_structure(mask)
    n_pat = len(pats)
    key = (blocks, n_pat, USE_BF16)
    if key not in _cache:
        _cache[key] = _build(blocks, n_pat)
    nc = _cache[key]

    xt = [np.ascontiguousarray(x[b].T).astype(NPDT) for b in range(B)]
    in_maps = []
    for c in range(NCORES):
        b, hg = c // HPC, c % HPC
        hs = slice(hg * DPC, (hg + 1) * DPC)
        wv_aug = np.zeros((D, VW), dtype=np.float32)
        bv_aug = np.zeros(VW, dtype=np.float32)
        for j in range(HPC):
            base = j * (HD + 1)
            rows = slice(hg * DPC + j * HD, hg * DPC + (j + 1) * HD)
            wv_aug[:, base:base + HD] = np.asarray(Wv)[rows, :].T
            bv_aug[base:base + HD] = np.asarray(bv)[rows]
            bv_aug[base + HD] = 1.0
        im = {
            "xt": xt[b],
            "wq": np.ascontiguousarray(np.asarray(Wq)[hs, :].T).astype(NPDT),
            "wk": np.ascontiguousarray(np.asarray(Wk)[hs, :].T).astype(NPDT),
            "wv": wv_aug.astype(NPDT),
            "wo": np.ascontiguousarray(np.asarray(Wo)[:, hs].T).astype(NPDT),
            "bq": np.ascontiguousarray(np.asarray(bq)[hs].reshape(2, 128).T),
            "bk": np.ascontiguousarray(np.asarray(bk)[hs].reshape(2, 128).T),
            "bv": bv_aug,
        }
        if n_pat:
            im["mp"] = np.concatenate(pats, axis=1).astype(NPDT)
        in_maps.append(im)

    res = run_bass_kernel_spmd(nc, in_maps, core_ids=list(range(NCORES))).results
    out = np.empty((B, S, D), dtype=np.float32)
    for b in range(B):
        acc = res[b * HPC]["out"].astype(np.float32)
        for g in range(1, HPC):
            acc = acc + res[b * HPC + g]["out"].astype(np.float32)
        out[b] = acc + np.asarray(bo, dtype=np.float32)[None, :]
    return out


# revision 7
# speedup vs baseline: 1.1589x; 1.1421x over previous
"""Causal self-attention on 8 trn2 NeuronCores.

Sharding: core c handles batch b = c//4 and heads 4*(c%4) .. 4*(c%4)+3
(data parallel on B, tensor parallel on the 16 heads). Each core computes
its 4 heads' attention plus the corresponding slice of the output
projection; the host sums the 4 partial projections per batch and adds bo.

On-chip layout is feature-major ("transposed"): qT/kT are [head_dim, seq],
scores are computed as sT[k, q] so the attention@v matmul needs no
transposes. The softmax denominator comes from an extra all-ones column
appended to Wv (so ctx PSUM row 64 accumulates sum_k exp). Normalization
broadcasts 1/denom across partitions via a tiny K=1 matmul.

Matmuls run in bfloat16 (hw-measured 2x faster than fp32/f32r for both
MATMUL and LDWEIGHTS on trn2). Causal masking is done with gpsimd
affine_select on the exp'd scores (keep iff q >= k, i.e. f - p - c >= 0
in the transposed block layout) instead of PE mask-add matmuls. Score
blocks are exp'd two at a time from a 2-bank PSUM tile to halve the
scalar-engine instruction count.
"""

import os
import sys

sys.path.insert(0, "/opt/trn_rl_repo")

import numpy as np
import ml_dtypes

import concourse.bass as bass
import concourse.tile as tile
from concourse import bacc, mybir
from concourse.bass_utils import run_bass_kernel_spmd

B, S, D, H = 2, 2048, 1024, 16
HD = D // H            # 64
NCORES = 8
HPC = 4                # heads per core
DPC = HPC * HD         # 256 feature dims per core
QT = 512               # q tile (free dim of score matmuls)
KC = 128               # k chunk (partition dim of transposed scores)
NQT = S // QT          # 4
NKC = S // KC          # 16
VW = HPC * (HD + 1)    # 260: v with ones column per head

F32 = mybir.dt.float32
F32R = mybir.dt.float32r
USE_BF16 = os.environ.get("KBF16", "1") == "1"
DT = mybir.dt.bfloat16 if USE_BF16 else F32R
NPDT = ml_dtypes.bfloat16 if USE_BF16 else np.float32

_cache = {}

# chunk kinds in the block structure
FULL = ("full",)


def _dram_ap(t, offset, dims):
    """dims: list of (elem_stride, n). Builds a raw AP on a dram tensor."""
    return bass.AP(tensor=t.tensor, offset=offset, ap=[list(d) for d in dims])


def _build(blocks, n_pat):
    """blocks: per q-tile, tuple of (ki, kind) chunks; kind is FULL,
    ('stair', c) with keep iff f - p - c >= 0, or ('mask', pat_idx)."""
    nc = bacc.Bacc(
        "TRN2",
        target_bir_lowering=False,
        debug=False,
        enable_asserts=False,
        num_devices=NCORES,
    )

    xt_d = nc.dram_tensor("xt", [D, S], DT, kind="ExternalInput").ap()
    wq_d = nc.dram_tensor("wq", [D, DPC], DT, kind="ExternalInput").ap()
    wk_d = nc.dram_tensor("wk", [D, DPC], DT, kind="ExternalInput").ap()
    wv_d = nc.dram_tensor("wv", [D, VW], DT, kind="ExternalInput").ap()
    wo_d = nc.dram_tensor("wo", [DPC, D], DT, kind="ExternalInput").ap()
    bq_d = nc.dram_tensor("bq", [128, 2], F32, kind="ExternalInput").ap()
    bk_d = nc.dram_tensor("bk", [128, 2], F32, kind="ExternalInput").ap()
    bv_d = nc.dram_tensor("bv", [VW], F32, kind="ExternalInput").ap()
    if n_pat:
        mp_d = nc.dram_tensor("mp", [128, n_pat * QT], DT, kind="ExternalInput").ap()
    out_d = nc.dram_tensor("out", [S, D], DT, kind="ExternalOutput").ap()

    with tile.TileContext(nc) as tc:
        with (
            tc.tile_pool(name="consts", bufs=1) as consts,
            tc.tile_pool(name="pe_ps", bufs=2, space="PSUM") as pe_ps,
            tc.tile_pool(name="st_ps", bufs=2, space="PSUM") as st_ps,
            tc.tile_pool(name="ctx_ps", bufs=2, space="PSUM") as ctx_ps,
            tc.tile_pool(name="work", bufs=8) as work,
            tc.tile_pool(name="norm", bufs=4) as norm,
            tc.tile_pool(name="ctxn", bufs=4) as ctxn,
            tc.tile_pool(name="stage", bufs=3) as stage,
        ):
            # ---- resident loads: one DMA descriptor per tensor, ordered so
            # the first q matmuls can start as early as possible ----
            wq_sb = consts.tile([128, 8, DPC], DT)
            nc.sync.dma_start(
                out=wq_sb,
                in_=_dram_ap(wq_d, 0, [(DPC, 128), (128 * DPC, 8), (1, DPC)]),
            )
            xtb = []
            for nb in range(NQT):
                xtb_t = consts.tile([128, 8, QT], DT, tag=f"xtb{nb}")
                xtb.append(xtb_t)
            nc.sync.dma_start(
                out=xtb[0],
                in_=_dram_ap(xt_d, 0, [(S, 128), (128 * S, 8), (1, QT)]),
            )
            wk_sb = consts.tile([128, 8, DPC], DT)
            wv_sb = consts.tile([128, 8, VW], DT)
            nc.sync.dma_start(
                out=wk_sb,
                in_=_dram_ap(wk_d, 0, [(DPC, 128), (128 * DPC, 8), (1, DPC)]),
            )
            nc.sync.dma_start(
                out=wv_sb,
                in_=_dram_ap(wv_d, 0, [(VW, 128), (128 * VW, 8), (1, VW)]),
            )
            bq_sb = consts.tile([128, 2], F32)
            bk_sb = consts.tile([128, 2], F32)
            bv_sb = consts.tile([128, VW], F32)
            nc.sync.dma_start(out=bq_sb, in_=bq_d)
            nc.sync.dma_start(out=bk_sb, in_=bk_d)
            nc.sync.dma_start(out=bv_sb, in_=_dram_ap(bv_d, 0, [(0, 128), (1, VW)]))
            for nb in range(1, NQT):
                nc.sync.dma_start(
                    out=xtb[nb],
                    in_=_dram_ap(xt_d, nb * QT, [(S, 128), (128 * S, 8), (1, QT)]),
                )
            wo_sb = consts.tile([128, 2, D], DT)
            nc.sync.dma_start(
                out=wo_sb, in_=_dram_ap(wo_d, 0, [(D, 128), (128 * D, 2), (1, D)])
            )
            if n_pat:
                mp_sb = consts.tile([128, n_pat, QT], DT)
                nc.sync.dma_start(
                    out=mp_sb,
                    in_=_dram_ap(
                        mp_d, 0, [(n_pat * QT, 128), (QT, n_pat), (1, QT)]
                    ),
                )

            ones_f = consts.tile([65, HD], F32)
            nc.vector.memset(ones_f, 1.0)
            ones_r = consts.tile([65, HD], F32R)
            nc.vector.tensor_copy(ones_r, ones_f)

            # ---- phase A: qT/kT = W @ xT, v = x @ Wv_aug (feature-major q/k) ----
            qt_sb = consts.tile([128, 2, S], DT)
            kt_sb = consts.tile([128, 2, S], DT)
            v_sb = consts.tile([128, NKC, VW], DT)

            def phase_a_block(n):
                for m in range(2):
                    ps = pe_ps.tile([128, QT], F32, tag="pe")
                    for kc in range(8):
                        nc.tensor.matmul(
                            ps,
                            wq_sb[:, kc, m * 128:(m + 1) * 128],
                            xtb[n][:, kc, :],
                            start=(kc == 0), stop=(kc == 7),
                        )
                    nc.vector.tensor_scalar_add(
                        qt_sb[:, m, n * QT:(n + 1) * QT], ps, bq_sb[:, m:m + 1]
                    )
                    ps = pe_ps.tile([128, QT], F32, tag="pe")
                    for kc in range(8):
                        nc.tensor.matmul(
                            ps,
                            wk_sb[:, kc, m * 128:(m + 1) * 128],
                            xtb[n][:, kc, :],
                            start=(kc == 0), stop=(kc == 7),
                        )
                    nc.vector.tensor_scalar_add(
                        kt_sb[:, m, n * QT:(n + 1) * QT], ps, bk_sb[:, m:m + 1]
                    )
                for sc in range(4 * n, 4 * n + 4):
                    ps = pe_ps.tile([128, VW], F32, tag="pe")
                    for kc in range(8):
                        nc.tensor.matmul(
                            ps,
                            xtb[n][:, kc, (sc % 4) * 128:(sc % 4 + 1) * 128],
                            wv_sb[:, kc, :],
                            start=(kc == 0), stop=(kc == 7),
                        )
                    nc.vector.tensor_add(v_sb[:, sc, :], ps, bv_sb)

            # ---- phase B: attention per q tile, software-pipelined ----
            # Two heads are processed together at score-pair granularity so
            # the PE always has independent matmuls to run while the
            # exp (scalar) -> causal-mask select (gpsimd) chain of the
            # previous pair is still in flight.
            cn_of = {}

            def emit_scores(qi, h, g):
                mc = h // 2
                fo = (h % 2) * HD
                qsl = slice(qi * QT, (qi + 1) * QT)
                st = st_ps.tile([128, len(g), QT], F32, tag="st")
                for si, (ki, kind) in enumerate(g):
                    nc.tensor.matmul(
                        st[:, si, :],
                        kt_sb[fo:fo + HD, mc, ki * 128:(ki + 1) * 128],
                        qt_sb[fo:fo + HD, mc, qsl],
                        start=True, stop=True,
                    )
                ex = work.tile([128, len(g), QT], DT)
                nc.scalar.activation(
                    out=ex, in_=st,
                    func=mybir.ActivationFunctionType.Exp, scale=0.125,
                )
                kinds = [kind for _, kind in g]
                if any(k[0] == "stair" for k in kinds) and all(
                    k[0] in ("stair", "full") for k in kinds
                ):
                    # keep iff f - p - c >= 0 per half (c=-512: keep all)
                    cs = [(k[1] if k[0] == "stair" else -QT) for k in kinds]
                    ex2 = work.tile([128, len(g), QT], DT, tag="ex2")
                    pat = (
                        [[cs[0] - cs[1], 2], [1, QT]]
                        if len(g) == 2 else [[1, QT]]
                    )
                    nc.gpsimd.affine_select(
                        out=ex2, in_=ex,
                        pattern=pat,
                        compare_op=mybir.AluOpType.is_ge,
                        fill=0.0, base=-cs[0], channel_multiplier=-1,
                    )
                    ex = ex2
                else:
                    for si, k in enumerate(kinds):
                        if k[0] == "full":
                            continue
                        ex2 = work.tile([128, QT], DT, tag="exm")
                        if k[0] == "stair":
                            nc.gpsimd.affine_select(
                                out=ex2, in_=ex[:, si, :],
                                pattern=[[1, QT]],
                                compare_op=mybir.AluOpType.is_ge,
                                fill=0.0, base=-k[1], channel_multiplier=-1,
                            )
                        else:
                            nc.gpsimd.tensor_mul(
                                ex2, ex[:, si, :], mp_sb[:, k[1], :]
                            )
                        nc.gpsimd.tensor_copy(ex[:, si, :], ex2)
                return ex

            def emit_ctx(ctx, h, g, ex, start, stop):
                for si, (ki, kind) in enumerate(g):
                    nc.tensor.matmul(
                        ctx,
                        v_sb[:, ki, h * (HD + 1):(h + 1) * (HD + 1)],
                        ex[:, si, :],
                        start=(start and si == 0),
                        stop=(stop and si == len(g) - 1),
                    )

            def attn_core(qi):
                cn0 = ctxn.tile([128, QT], DT, tag="cn0")
                cn1 = ctxn.tile([128, QT], DT, tag="cn1")
                cn = [cn0, cn1]
                cn_of[qi] = cn
                chunks = blocks[qi]
                groups = [chunks[i:i + 2] for i in range(0, len(chunks), 2)]
                ng = len(groups)
                deferred = []

                def norm_pe(h, ctx, dn_sb):
                    even = (h % 2 == 0)
                    mc = h // 2
                    bc = pe_ps.tile([HD, QT], F32, tag="pe")
                    nc.tensor.matmul(
                        bc, ones_r[HD:HD + 1, :], dn_sb[HD:HD + 1, :],
                        start=True, stop=True,
                    )
                    rc = norm.tile([HD, QT], F32, tag="rc")
                    nc.vector.reciprocal_approx_fast(out=rc, in_=bc)
                    if even:
                        nc.vector.tensor_mul(cn[mc][0:HD, :], ctx[0:HD, :], rc)
                    else:
                        tmp2 = norm.tile([HD, QT], DT, tag="tmp2")
                        nc.vector.tensor_mul(tmp2, ctx[0:HD, :], rc)
                        nc.sync.dma_start(out=cn[mc][HD:2 * HD, :], in_=tmp2)

                for hp, heads in enumerate(((2, 3), (0, 1))):
                    ctxs = {
                        h: ctx_ps.tile(
                            [HD + 1, QT], F32, tag="ctx", name=f"ctx{h}"
                        )
                        for h in heads
                    }
                    exs = {}
                    for g in range(ng):
                        for h in heads:
                            exs[(h, g)] = emit_scores(qi, h, groups[g])
                        if hp == 1 and g == 1 and deferred:
                            # previous head-pair's norm PE work, emitted here
                            # so its denominator copy has had time to land
                            for fn in deferred:
                                fn()
                            deferred = []
                        if g >= 1:
                            for h in heads:
                                emit_ctx(ctxs[h], h, groups[g - 1],
                                         exs.pop((h, g - 1)),
                                         start=(g == 1), stop=False)
                    for h in heads:
                        emit_ctx(ctxs[h], h, groups[ng - 1],
                                 exs.pop((h, ng - 1)),
                                 start=(ng == 1), stop=True)
                    for h in heads:
                        dn_sb = norm.tile([HD + 1, QT], F32R, tag="dn")
                        with nc.allow_low_precision(reason="f32r bcast operand"):
                            nc.vector.tensor_copy(
                                dn_sb[HD:HD + 1, :], ctxs[h][HD:HD + 1, :]
                            )
                        deferred.append(
                            (lambda h=h, c=ctxs[h], d=dn_sb: norm_pe(h, c, d))
                        )
                for fn in deferred:
                    fn()

            def attn_out(qi):
                cn = cn_of.pop(qi)
                for qc in range(4):
                    so = stage.tile([128, 2, QT], DT)
                    for ne in range(2):
                        ps = pe_ps.tile([128, QT], F32, tag="pe")
                        for cc in (1, 0):
                            nc.tensor.matmul(
                                ps,
                                cn[cc][:, qc * 128:(qc + 1) * 128],
                                wo_sb[:, cc, ne * QT:(ne + 1) * QT],
                                start=(cc == 1), stop=(cc == 0),
                            )
                        nc.vector.tensor_copy(so[:, ne, :], ps)
                    nc.sync.dma_start(
                        out=out_d[qi * QT + qc * 128: qi * QT + (qc + 1) * 128, :],
                        in_=so,
                    )

            # interleave: attention scores/ctx for a q-tile right after the
            # phase-A block that completes its inputs; its output projection
            # is deferred until after the NEXT phase-A block so the PE has
            # work while the last softmax normalizations drain.
            ready_at = [max(ki for ki, _ in blocks[qi]) // 4 for qi in range(NQT)]
            pending_out = []
            for n in range(NQT):
                phase_a_block(n)
                for qi in pending_out:
                    attn_out(qi)
                pending_out = []
                for qi in range(NQT):
                    if ready_at[qi] == n:
                        attn_core(qi)
                        pending_out.append(qi)
            for qi in pending_out:
                attn_out(qi)

    nc.compile()
    return nc


def _block_structure(mask):
    """Classify [QT x KC] score blocks from the runtime mask (mask[q, k])."""
    allowed = ~np.isneginf(np.asarray(mask, dtype=np.float32))
    pats = []
    pat_idx = {}
    blocks = []
    for qi in range(NQT):
        row = []
        for ki in range(NKC):
            sub = allowed[qi * QT:(qi + 1) * QT, ki * KC:(ki + 1) * KC]
            if not sub.any():
                continue
            if sub.all():
                row.append((ki, FULL))
                continue
            # staircase: keep (p, f) iff f >= p + c, in transposed [k, q] view
            subT = sub.T  # [128 k, 512 q]
            c = None
            for cand in range(-QT + 1, KC + QT):
                ref = (np.arange(QT)[None, :] >= np.arange(KC)[:, None] + cand)
                if np.array_equal(subT, ref):
                    c = cand
                    break
            if c is not None:
                row.append((ki, ("stair", c)))
            else:
                pat = np.ascontiguousarray(
                    np.where(subT, 1.0, 0.0).astype(np.float32)
                )  # [128, 512] multiplicative mask
                key = pat.tobytes()
                if key not in pat_idx:
                    pat_idx[key] = len(pats)
                    pats.append(pat)
                row.append((ki, ("mask", pat_idx[key])))
        blocks.append(tuple(row))
    return tuple(blocks), pats


def kernel(x, mask, Wq, bq, Wk, bk, Wv, bv, Wo, bo):
    x = np.asarray(x, dtype=np.float32)
    blocks, pats = _block_structure(mask)
    n_pat = len(pats)
    key = (blocks, n_pat, USE_BF16)
    if key not in _cache:
        _cache[key] = _build(blocks, n_pat)
    nc = _cache[key]

    xt = [np.ascontiguousarray(x[b].T).astype(NPDT) for b in range(B)]
    in_maps = []
    for c in range(NCORES):
        b, hg = c // HPC, c % HPC
        hs = slice(hg * DPC, (hg + 1) * DPC)
        wv_aug = np.zeros((D, VW), dtype=np.float32)
        bv_aug = np.zeros(VW, dtype=np.float32)
        for j in range(HPC):
            base = j * (HD + 1)
            rows = slice(hg * DPC + j * HD, hg * DPC + (j + 1) * HD)
            wv_aug[:, base:base + HD] = np.asarray(Wv)[rows, :].T
            bv_aug[base:base + HD] = np.asarray(bv)[rows]
            bv_aug[base + HD] = 1.0
        im = {
            "xt": xt[b],
            "wq": np.ascontiguousarray(np.asarray(Wq)[hs, :].T).astype(NPDT),
            "wk": np.ascontiguousarray(np.asarray(Wk)[hs, :].T).astype(NPDT),
            "wv": wv_aug.astype(NPDT),
            "wo": np.ascontiguousarray(np.asarray(Wo)[:, hs].T).astype(NPDT),
            "bq": np.ascontiguousarray(np.asarray(bq)[hs].reshape(2, 128).T),
            "bk": np.ascontiguousarray(np.asarray(bk)[hs].reshape(2, 128).T),
            "bv": bv_aug,
        }
        if n_pat:
            im["mp"] = np.concatenate(pats, axis=1).astype(NPDT)
        in_maps.append(im)

    res = run_bass_kernel_spmd(nc, in_maps, core_ids=list(range(NCORES))).results
    out = np.empty((B, S, D), dtype=np.float32)
    for b in range(B):
        acc = res[b * HPC]["out"].astype(np.float32)
        for g in range(1, HPC):
            acc = acc + res[b * HPC + g]["out"].astype(np.float32)
        out[b] = acc + np.asarray(bo, dtype=np.float32)[None, :]
    return out


# revision 12
# speedup vs baseline: 1.2033x; 1.0383x over previous
"""Causal self-attention on 8 trn2 NeuronCores.

Sharding: core c handles batch b = c//4 and heads 4*(c%4) .. 4*(c%4)+3
(data parallel on B, tensor parallel on the 16 heads). Each core computes
its 4 heads' attention plus the corresponding slice of the output
projection; the host sums the 4 partial projections per batch and adds bo.

On-chip layout is feature-major ("transposed"): qT/kT are [head_dim, seq],
scores are computed as sT[k, q] so the attention@v matmul needs no
transposes. The softmax denominator comes from an extra all-ones column
appended to Wv (so ctx PSUM row 64 accumulates sum_k exp). Normalization
broadcasts 1/denom across partitions via a tiny K=1 matmul.

Matmuls run in bfloat16 (hw-measured 2x faster than fp32/f32r for both
MATMUL and LDWEIGHTS on trn2). Causal masking is done with gpsimd
affine_select on the exp'd scores (keep iff q >= k, i.e. f - p - c >= 0
in the transposed block layout) instead of PE mask-add matmuls. Score
blocks are exp'd two at a time from a 2-bank PSUM tile to halve the
scalar-engine instruction count.
"""

import os
import sys

sys.path.insert(0, "/opt/trn_rl_repo")

import numpy as np
import ml_dtypes

import concourse.bass as bass
import concourse.tile as tile
from concourse import bacc, mybir
from concourse.bass_utils import run_bass_kernel_spmd

B, S, D, H = 2, 2048, 1024, 16
HD = D // H            # 64
NCORES = 8
HPC = 4                # heads per core
DPC = HPC * HD         # 256 feature dims per core
QT = 512               # q tile (free dim of score matmuls)
KC = 128               # k chunk (partition dim of transposed scores)
NQT = S // QT          # 4
NKC = S // KC          # 16
VW = HPC * (HD + 1)    # 260: v with ones column per head

F32 = mybir.dt.float32
F32R = mybir.dt.float32r
USE_BF16 = os.environ.get("KBF16", "1") == "1"
DT = mybir.dt.bfloat16 if USE_BF16 else F32R
NPDT = ml_dtypes.bfloat16 if USE_BF16 else np.float32

_cache = {}

# chunk kinds in the block structure
FULL = ("full",)


def _dram_ap(t, offset, dims):
    """dims: list of (elem_stride, n). Builds a raw AP on a dram tensor."""
    return bass.AP(tensor=t.tensor, offset=offset, ap=[list(d) for d in dims])


def _build(blocks, n_pat):
    """blocks: per q-tile, tuple of (ki, kind) chunks; kind is FULL,
    ('stair', c) with keep iff f - p - c >= 0, or ('mask', pat_idx)."""
    nc = bacc.Bacc(
        "TRN2",
        target_bir_lowering=False,
        debug=False,
        enable_asserts=False,
        num_devices=NCORES,
    )

    xt_d = nc.dram_tensor("xt", [D, S], DT, kind="ExternalInput").ap()
    wq_d = nc.dram_tensor("wq", [D, DPC], DT, kind="ExternalInput").ap()
    wk_d = nc.dram_tensor("wk", [D, DPC], DT, kind="ExternalInput").ap()
    wv_d = nc.dram_tensor("wv", [D, VW], DT, kind="ExternalInput").ap()
    wo_d = nc.dram_tensor("wo", [DPC, D], DT, kind="ExternalInput").ap()
    bq_d = nc.dram_tensor("bq", [128, 2], F32, kind="ExternalInput").ap()
    bk_d = nc.dram_tensor("bk", [128, 2], F32, kind="ExternalInput").ap()
    bv_d = nc.dram_tensor("bv", [VW], F32, kind="ExternalInput").ap()
    if n_pat:
        mp_d = nc.dram_tensor("mp", [128, n_pat * QT], DT, kind="ExternalInput").ap()
    out_d = nc.dram_tensor("out", [S, D], DT, kind="ExternalOutput").ap()

    with tile.TileContext(nc) as tc:
        with (
            tc.tile_pool(name="consts", bufs=1) as consts,
            tc.tile_pool(name="pe_ps", bufs=2, space="PSUM") as pe_ps,
            tc.tile_pool(name="st_ps", bufs=2, space="PSUM") as st_ps,
            tc.tile_pool(name="ctx_ps", bufs=2, space="PSUM") as ctx_ps,
            tc.tile_pool(name="work", bufs=8) as work,
            tc.tile_pool(name="norm", bufs=4) as norm,
            tc.tile_pool(name="ctxn", bufs=4) as ctxn,
            tc.tile_pool(name="stage", bufs=3) as stage,
        ):
            # ---- resident loads: one DMA descriptor per tensor, ordered so
            # the first q matmuls can start as early as possible ----
            wq_sb = consts.tile([128, 8, DPC], DT)
            xtb = []
            for nb in range(NQT):
                xtb_t = consts.tile([128, 8, QT], DT, tag=f"xtb{nb}")
                xtb.append(xtb_t)
            # first-needed tensors split in halves so compute starts sooner
            for ha in range(2):
                nc.sync.dma_start(
                    out=wq_sb[:, ha * 4:(ha + 1) * 4, :],
                    in_=_dram_ap(
                        wq_d, ha * 4 * 128 * DPC,
                        [(DPC, 128), (128 * DPC, 4), (1, DPC)],
                    ),
                )
                nc.sync.dma_start(
                    out=xtb[0][:, ha * 4:(ha + 1) * 4, :],
                    in_=_dram_ap(
                        xt_d, ha * 4 * 128 * S,
                        [(S, 128), (128 * S, 4), (1, QT)],
                    ),
                )
            wk_sb = consts.tile([128, 8, DPC], DT)
            wv_sb = consts.tile([128, 8, VW], DT)
            nc.sync.dma_start(
                out=wk_sb,
                in_=_dram_ap(wk_d, 0, [(DPC, 128), (128 * DPC, 8), (1, DPC)]),
            )
            nc.sync.dma_start(
                out=wv_sb,
                in_=_dram_ap(wv_d, 0, [(VW, 128), (128 * VW, 8), (1, VW)]),
            )
            bq_sb = consts.tile([128, 2], F32)
            bk_sb = consts.tile([128, 2], F32)
            bv_sb = consts.tile([128, VW], F32)
            nc.sync.dma_start(out=bq_sb, in_=bq_d)
            nc.sync.dma_start(out=bk_sb, in_=bk_d)
            nc.sync.dma_start(out=bv_sb, in_=_dram_ap(bv_d, 0, [(0, 128), (1, VW)]))
            for nb in range(1, NQT):
                nc.sync.dma_start(
                    out=xtb[nb],
                    in_=_dram_ap(xt_d, nb * QT, [(S, 128), (128 * S, 8), (1, QT)]),
                )
            wo_sb = consts.tile([128, 2, D], DT)
            nc.sync.dma_start(
                out=wo_sb, in_=_dram_ap(wo_d, 0, [(D, 128), (128 * D, 2), (1, D)])
            )
            if n_pat:
                mp_sb = consts.tile([128, n_pat, QT], DT)
                nc.sync.dma_start(
                    out=mp_sb,
                    in_=_dram_ap(
                        mp_d, 0, [(n_pat * QT, 128), (QT, n_pat), (1, QT)]
                    ),
                )

            ones_f = consts.tile([65, HD], F32)
            nc.vector.memset(ones_f, 1.0)
            ones_r = consts.tile([65, HD], F32R)
            nc.vector.tensor_copy(ones_r, ones_f)

            # ---- phase A: qT/kT = W @ xT, v = x @ Wv_aug (feature-major q/k) ----
            qt_sb = consts.tile([128, 2, S], DT)
            kt_sb = consts.tile([128, 2, S], DT)
            v_sb = consts.tile([128, NKC, VW], DT)

            def phase_a_block(n, after_first=None):
                for m in range(2):
                    ps = pe_ps.tile([128, QT], F32, tag="pe")
                    for kc in range(8):
                        nc.tensor.matmul(
                            ps,
                            wq_sb[:, kc, m * 128:(m + 1) * 128],
                            xtb[n][:, kc, :],
                            start=(kc == 0), stop=(kc == 7),
                        )
                    nc.vector.tensor_scalar_add(
                        qt_sb[:, m, n * QT:(n + 1) * QT], ps, bq_sb[:, m:m + 1]
                    )
                    if m == 0 and after_first:
                        for fn in after_first:
                            fn()
                        after_first.clear()
                    ps = pe_ps.tile([128, QT], F32, tag="pe")
                    for kc in range(8):
                        nc.tensor.matmul(
                            ps,
                            wk_sb[:, kc, m * 128:(m + 1) * 128],
                            xtb[n][:, kc, :],
                            start=(kc == 0), stop=(kc == 7),
                        )
                    nc.vector.tensor_scalar_add(
                        kt_sb[:, m, n * QT:(n + 1) * QT], ps, bk_sb[:, m:m + 1]
                    )
                for sc in range(4 * n, 4 * n + 4):
                    ps = pe_ps.tile([128, VW], F32, tag="pe")
                    for kc in range(8):
                        nc.tensor.matmul(
                            ps,
                            xtb[n][:, kc, (sc % 4) * 128:(sc % 4 + 1) * 128],
                            wv_sb[:, kc, :],
                            start=(kc == 0), stop=(kc == 7),
                        )
                    nc.vector.tensor_add(v_sb[:, sc, :], ps, bv_sb)

            # ---- phase B: attention per q tile, software-pipelined ----
            # Two heads are processed together at score-pair granularity so
            # the PE always has independent matmuls to run while the
            # exp (scalar) -> causal-mask select (gpsimd) chain of the
            # previous pair is still in flight.
            cn_of = {}

            def emit_scores(qi, h, g):
                mc = h // 2
                fo = (h % 2) * HD
                qsl = slice(qi * QT, (qi + 1) * QT)
                st = st_ps.tile([128, len(g), QT], F32, tag="st")
                for si, (ki, kind) in enumerate(g):
                    nc.tensor.matmul(
                        st[:, si, :],
                        kt_sb[fo:fo + HD, mc, ki * 128:(ki + 1) * 128],
                        qt_sb[fo:fo + HD, mc, qsl],
                        start=True, stop=True,
                    )
                ex = work.tile([128, len(g), QT], DT)
                nc.scalar.activation(
                    out=ex, in_=st,
                    func=mybir.ActivationFunctionType.Exp, scale=0.125,
                )
                kinds = [kind for _, kind in g]
                if any(k[0] == "stair" for k in kinds) and all(
                    k[0] in ("stair", "full") for k in kinds
                ):
                    # keep iff f - p - c >= 0 per half (c=-512: keep all)
                    cs = [(k[1] if k[0] == "stair" else -QT) for k in kinds]
                    ex2 = work.tile([128, len(g), QT], DT, tag="ex2")
                    pat = (
                        [[cs[0] - cs[1], 2], [1, QT]]
                        if len(g) == 2 else [[1, QT]]
                    )
                    nc.gpsimd.affine_select(
                        out=ex2, in_=ex,
                        pattern=pat,
                        compare_op=mybir.AluOpType.is_ge,
                        fill=0.0, base=-cs[0], channel_multiplier=-1,
                    )
                    ex = ex2
                else:
                    for si, k in enumerate(kinds):
                        if k[0] == "full":
                            continue
                        ex2 = work.tile([128, QT], DT, tag="exm")
                        if k[0] == "stair":
                            nc.gpsimd.affine_select(
                                out=ex2, in_=ex[:, si, :],
                                pattern=[[1, QT]],
                                compare_op=mybir.AluOpType.is_ge,
                                fill=0.0, base=-k[1], channel_multiplier=-1,
                            )
                        else:
                            nc.gpsimd.tensor_mul(
                                ex2, ex[:, si, :], mp_sb[:, k[1], :]
                            )
                        nc.gpsimd.tensor_copy(ex[:, si, :], ex2)
                return ex

            def emit_ctx(ctx, h, g, ex, start, stop):
                for si, (ki, kind) in enumerate(g):
                    nc.tensor.matmul(
                        ctx,
                        v_sb[:, ki, h * (HD + 1):(h + 1) * (HD + 1)],
                        ex[:, si, :],
                        start=(start and si == 0),
                        stop=(stop and si == len(g) - 1),
                    )

            def attn_core(qi):
                cn0 = ctxn.tile([128, QT], DT, tag="cn0")
                cn1 = ctxn.tile([128, QT], DT, tag="cn1")
                cn = [cn0, cn1]
                cn_of[qi] = cn
                chunks = blocks[qi]
                groups = [chunks[i:i + 2] for i in range(0, len(chunks), 2)]
                ng = len(groups)
                deferred = []

                def norm_pe(h, ctx, dn_sb):
                    even = (h % 2 == 0)
                    mc = h // 2
                    bc = pe_ps.tile([HD, QT], F32, tag="pe")
                    nc.tensor.matmul(
                        bc, ones_r[HD:HD + 1, :], dn_sb[HD:HD + 1, :],
                        start=True, stop=True,
                    )
                    rc = norm.tile([HD, QT], F32, tag="rc")
                    nc.vector.reciprocal_approx_fast(out=rc, in_=bc)
                    if even:
                        nc.vector.tensor_mul(cn[mc][0:HD, :], ctx[0:HD, :], rc)
                    else:
                        tmp2 = norm.tile([HD, QT], DT, tag="tmp2")
                        nc.vector.tensor_mul(tmp2, ctx[0:HD, :], rc)
                        nc.sync.dma_start(out=cn[mc][HD:2 * HD, :], in_=tmp2)

                for hp, heads in enumerate(((2, 3), (0, 1))):
                    ctxs = {
                        h: ctx_ps.tile(
                            [HD + 1, QT], F32, tag="ctx", name=f"ctx{h}"
                        )
                        for h in heads
                    }
                    exs = {}
                    for g in range(ng):
                        for h in heads:
                            exs[(h, g)] = emit_scores(qi, h, groups[g])
                        if hp == 1 and g == 1 and deferred:
                            # previous head-pair's norm PE work, emitted here
                            # so its denominator copy has had time to land
                            for fn in deferred:
                                fn()
                            deferred = []
                        if g >= 1:
                            for h in heads:
                                emit_ctx(ctxs[h], h, groups[g - 1],
                                         exs.pop((h, g - 1)),
                                         start=(g == 1), stop=False)
                    for h in heads:
                        emit_ctx(ctxs[h], h, groups[ng - 1],
                                 exs.pop((h, ng - 1)),
                                 start=(ng == 1), stop=True)
                    for h in heads:
                        dn_sb = norm.tile([HD + 1, QT], F32R, tag="dn")
                        with nc.allow_low_precision(reason="f32r bcast operand"):
                            nc.vector.tensor_copy(
                                dn_sb[HD:HD + 1, :], ctxs[h][HD:HD + 1, :]
                            )
                        deferred.append(
                            (lambda h=h, c=ctxs[h], d=dn_sb: norm_pe(h, c, d))
                        )
                return deferred

            def attn_out(qi):
                cn = cn_of.pop(qi)
                for qc in range(4):
                    so = stage.tile([128, 2, QT], DT)
                    for ne in range(2):
                        ps = pe_ps.tile([128, QT], F32, tag="pe")
                        for cc in (1, 0):
                            nc.tensor.matmul(
                                ps,
                                cn[cc][:, qc * 128:(qc + 1) * 128],
                                wo_sb[:, cc, ne * QT:(ne + 1) * QT],
                                start=(cc == 1), stop=(cc == 0),
                            )
                        nc.vector.tensor_copy(so[:, ne, :], ps)
                    nc.sync.dma_start(
                        out=out_d[qi * QT + qc * 128: qi * QT + (qc + 1) * 128, :],
                        in_=so,
                    )

            # interleave: attention scores/ctx for a q-tile right after the
            # phase-A block that completes its inputs; its output projection
            # is deferred until after the NEXT phase-A block so the PE has
            # work while the last softmax normalizations drain.
            ready_at = [max(ki for ki, _ in blocks[qi]) // 4 for qi in range(NQT)]
            pending_out = []
            pending_norms = []
            for n in range(NQT):
                phase_a_block(n, after_first=pending_norms)
                for qi in pending_out:
                    attn_out(qi)
                pending_out = []
                for qi in range(NQT):
                    if ready_at[qi] == n:
                        for fn in pending_norms:
                            fn()
                        pending_norms = attn_core(qi)
                        pending_out.append(qi)
            for fn in pending_norms:
                fn()
            for qi in pending_out:
                attn_out(qi)

    nc.compile()
    return nc


def _block_structure(mask):
    """Classify [QT x KC] score blocks from the runtime mask (mask[q, k])."""
    allowed = ~np.isneginf(np.asarray(mask, dtype=np.float32))
    pats = []
    pat_idx = {}
    blocks = []
    for qi in range(NQT):
        row = []
        for ki in range(NKC):
            sub = allowed[qi * QT:(qi + 1) * QT, ki * KC:(ki + 1) * KC]
            if not sub.any():
                continue
            if sub.all():
                row.append((ki, FULL))
                continue
            # staircase: keep (p, f) iff f >= p + c, in transposed [k, q] view
            subT = sub.T  # [128 k, 512 q]
            c = None
            for cand in range(-QT + 1, KC + QT):
                ref = (np.arange(QT)[None, :] >= np.arange(KC)[:, None] + cand)
                if np.array_equal(subT, ref):
                    c = cand
                    break
            if c is not None:
                row.append((ki, ("stair", c)))
            else:
                pat = np.ascontiguousarray(
                    np.where(subT, 1.0, 0.0).astype(np.float32)
                )  # [128, 512] multiplicative mask
                key = pat.tobytes()
                if key not in pat_idx:
                    pat_idx[key] = len(pats)
                    pats.append(pat)
                row.append((ki, ("mask", pat_idx[key])))
        blocks.append(tuple(row))
    return tuple(blocks), pats


def kernel(x, mask, Wq, bq, Wk, bk, Wv, bv, Wo, bo):
    x = np.asarray(x, dtype=np.float32)
    blocks, pats = _block_structure(mask)
    n_pat = len(pats)
    key = (blocks, n_pat, USE_BF16)
    if key not in _cache:
        _cache[key] = _build(blocks, n_pat)
    nc = _cache[key]

    xt = [np.ascontiguousarray(x[b].T).astype(NPDT) for b in range(B)]
    in_maps = []
    for c in range(NCORES):
        b, hg = c // HPC, c % HPC
        hs = slice(hg * DPC, (hg + 1) * DPC)
        wv_aug = np.zeros((D, VW), dtype=np.float32)
        bv_aug = np.zeros(VW, dtype=np.float32)
        for j in range(HPC):
            base = j * (HD + 1)
            rows = slice(hg * DPC + j * HD, hg * DPC + (j + 1) * HD)
            wv_aug[:, base:base + HD] = np.asarray(Wv)[rows, :].T
            bv_aug[base:base + HD] = np.asarray(bv)[rows]
            bv_aug[base + HD] = 1.0
        im = {
            "xt": xt[b],
            "wq": np.ascontiguousarray(np.asarray(Wq)[hs, :].T).astype(NPDT),
            "wk": np.ascontiguousarray(np.asarray(Wk)[hs, :].T).astype(NPDT),
            "wv": wv_aug.astype(NPDT),
            "wo": np.ascontiguousarray(np.asarray(Wo)[:, hs].T).astype(NPDT),
            "bq": np.ascontiguousarray(np.asarray(bq)[hs].reshape(2, 128).T),
            "bk": np.ascontiguousarray(np.asarray(bk)[hs].reshape(2, 128).T),
            "bv": bv_aug,
        }
        if n_pat:
            im["mp"] = np.concatenate(pats, axis=1).astype(NPDT)
        in_maps.append(im)

    res = run_bass_kernel_spmd(nc, in_maps, core_ids=list(range(NCORES))).results
    out = np.empty((B, S, D), dtype=np.float32)
    for b in range(B):
        acc = res[b * HPC]["out"].astype(np.float32)
        for g in range(1, HPC):
            acc = acc + res[b * HPC + g]["out"].astype(np.float32)
        out[b] = acc + np.asarray(bo, dtype=np.float32)[None, :]
    return out


# revision 18
# speedup vs baseline: 1.2932x; 1.0747x over previous
"""Causal self-attention on 8 trn2 NeuronCores.

Sharding: core c handles batch b = c//4 and heads 4*(c%4) .. 4*(c%4)+3
(data parallel on B, tensor parallel on the 16 heads). Each core computes
its 4 heads' attention plus the corresponding slice of the output
projection; the host sums the 4 partial projections per batch and adds bo.

On-chip layout is feature-major ("transposed"): qT/kT are [head_dim, seq],
scores are computed as sT[k, q] so the attention@v matmul needs no
transposes. The softmax denominator comes from an extra all-ones column
appended to Wv (so ctx PSUM row 64 accumulates sum_k exp). Normalization
broadcasts 1/denom across partitions via a tiny K=1 matmul.

Matmuls run in bfloat16 (hw-measured 2x faster than fp32/f32r for both
MATMUL and LDWEIGHTS on trn2). Causal masking is done with gpsimd
affine_select on the exp'd scores (keep iff q >= k, i.e. f - p - c >= 0
in the transposed block layout) instead of PE mask-add matmuls. Score
blocks are exp'd two at a time from a 2-bank PSUM tile to halve the
scalar-engine instruction count.
"""

import os
import sys

sys.path.insert(0, "/opt/trn_rl_repo")

import numpy as np
import ml_dtypes

import concourse.bass as bass
import concourse.tile as tile
from concourse import bacc, mybir
from concourse.bass_utils import run_bass_kernel_spmd

B, S, D, H = 2, 2048, 1024, 16
HD = D // H            # 64
NCORES = 8
HPC = 4                # heads per core
DPC = HPC * HD         # 256 feature dims per core
QT = 512               # q tile (free dim of score matmuls)
KC = 128               # k chunk (partition dim of transposed scores)
NQT = S // QT          # 4
NKC = S // KC          # 16
VW = HPC * (HD + 1)    # 260: v with ones column per head

F32 = mybir.dt.float32
F32R = mybir.dt.float32r
USE_BF16 = os.environ.get("KBF16", "1") == "1"
DT = mybir.dt.bfloat16 if USE_BF16 else F32R
NPDT = ml_dtypes.bfloat16 if USE_BF16 else np.float32

_cache = {}

# chunk kinds in the block structure
FULL = ("full",)


def _dram_ap(t, offset, dims):
    """dims: list of (elem_stride, n). Builds a raw AP on a dram tensor."""
    return bass.AP(tensor=t.tensor, offset=offset, ap=[list(d) for d in dims])


def _build(blocks, n_pat):
    """blocks: per q-tile, tuple of (ki, kind) chunks; kind is FULL,
    ('stair', c) with keep iff f - p - c >= 0, or ('mask', pat_idx)."""
    nc = bacc.Bacc(
        "TRN2",
        target_bir_lowering=False,
        debug=False,
        enable_asserts=False,
        num_devices=NCORES,
    )

    xt_d = nc.dram_tensor("xt", [D, S], DT, kind="ExternalInput").ap()
    wq_d = nc.dram_tensor("wq", [D, DPC], DT, kind="ExternalInput").ap()
    wk_d = nc.dram_tensor("wk", [D, DPC], DT, kind="ExternalInput").ap()
    wv_d = nc.dram_tensor("wv", [D, VW], DT, kind="ExternalInput").ap()
    wo_d = nc.dram_tensor("wo", [DPC, D], DT, kind="ExternalInput").ap()
    bq_d = nc.dram_tensor("bq", [128, 2], F32, kind="ExternalInput").ap()
    bk_d = nc.dram_tensor("bk", [128, 2], F32, kind="ExternalInput").ap()
    bv_d = nc.dram_tensor("bv", [VW], F32, kind="ExternalInput").ap()
    if n_pat:
        mp_d = nc.dram_tensor("mp", [128, n_pat * QT], DT, kind="ExternalInput").ap()
    out_d = nc.dram_tensor("out", [S, D], DT, kind="ExternalOutput").ap()

    with tile.TileContext(nc) as tc:
        with (
            tc.tile_pool(name="consts", bufs=1) as consts,
            tc.tile_pool(name="pe_ps", bufs=2, space="PSUM") as pe_ps,
            tc.tile_pool(name="st_ps", bufs=2, space="PSUM") as st_ps,
            tc.tile_pool(name="ctx_ps", bufs=2, space="PSUM") as ctx_ps,
            tc.tile_pool(name="work", bufs=8) as work,
            tc.tile_pool(name="norm", bufs=4) as norm,
            tc.tile_pool(name="ctxn", bufs=4) as ctxn,
            tc.tile_pool(name="stage", bufs=3) as stage,
        ):
            # ---- resident loads: one DMA descriptor per tensor, ordered so
            # the first q matmuls can start as early as possible ----
            wq_sb = consts.tile([128, 8, DPC], DT)
            xtb = []
            for nb in range(NQT):
                xtb_t = consts.tile([128, 8, QT], DT, tag=f"xtb{nb}")
                xtb.append(xtb_t)
            # first-needed tensors split in halves so compute starts sooner
            for ha in range(2):
                nc.sync.dma_start(
                    out=wq_sb[:, ha * 4:(ha + 1) * 4, :],
                    in_=_dram_ap(
                        wq_d, ha * 4 * 128 * DPC,
                        [(DPC, 128), (128 * DPC, 4), (1, DPC)],
                    ),
                )
                nc.sync.dma_start(
                    out=xtb[0][:, ha * 4:(ha + 1) * 4, :],
                    in_=_dram_ap(
                        xt_d, ha * 4 * 128 * S,
                        [(S, 128), (128 * S, 4), (1, QT)],
                    ),
                )
            wk_sb = consts.tile([128, 8, DPC], DT)
            wv_sb = consts.tile([128, 8, VW], DT)
            nc.sync.dma_start(
                out=wk_sb,
                in_=_dram_ap(wk_d, 0, [(DPC, 128), (128 * DPC, 8), (1, DPC)]),
            )
            nc.sync.dma_start(
                out=wv_sb,
                in_=_dram_ap(wv_d, 0, [(VW, 128), (128 * VW, 8), (1, VW)]),
            )
            bq_sb = consts.tile([128, 2], F32)
            bk_sb = consts.tile([128, 2], F32)
            bv_sb = consts.tile([128, VW], F32)
            nc.sync.dma_start(out=bq_sb, in_=bq_d)
            nc.sync.dma_start(out=bk_sb, in_=bk_d)
            nc.sync.dma_start(out=bv_sb, in_=_dram_ap(bv_d, 0, [(0, 128), (1, VW)]))
            for nb in range(1, NQT):
                nc.sync.dma_start(
                    out=xtb[nb],
                    in_=_dram_ap(xt_d, nb * QT, [(S, 128), (128 * S, 8), (1, QT)]),
                )
            wo_sb = consts.tile([128, 2, D], DT)
            nc.sync.dma_start(
                out=wo_sb, in_=_dram_ap(wo_d, 0, [(D, 128), (128 * D, 2), (1, D)])
            )
            if n_pat:
                mp_sb = consts.tile([128, n_pat, QT], DT)
                nc.sync.dma_start(
                    out=mp_sb,
                    in_=_dram_ap(
                        mp_d, 0, [(n_pat * QT, 128), (QT, n_pat), (1, QT)]
                    ),
                )

            ones_f = consts.tile([65, HD], F32)
            nc.vector.memset(ones_f, 1.0)
            ones_r = consts.tile([65, HD], F32R)
            nc.vector.tensor_copy(ones_r, ones_f)

            # ---- phase A: qT/kT = W @ xT, v = x @ Wv_aug (feature-major q/k) ----
            qt_sb = consts.tile([128, 2, S], DT)
            kt_sb = consts.tile([128, 2, S], DT)
            v_sb = consts.tile([128, NKC, VW], DT)

            def emit_qk_group(n, m, which):
                w_sb, b_sb, t_sb = (
                    (wq_sb, bq_sb, qt_sb) if which == "q" else (wk_sb, bk_sb, kt_sb)
                )
                ps = pe_ps.tile([128, QT], F32, tag="pe", name="psqk")
                for kc in range(8):
                    nc.tensor.matmul(
                        ps,
                        w_sb[:, kc, m * 128:(m + 1) * 128],
                        xtb[n][:, kc, :],
                        start=(kc == 0), stop=(kc == 7),
                    )
                nc.vector.tensor_scalar_add(
                    t_sb[:, m, n * QT:(n + 1) * QT], ps, b_sb[:, m:m + 1]
                )

            def emit_v_group(n, sc):
                ps = pe_ps.tile([128, VW], F32, tag="pe", name="psv")
                for kc in range(8):
                    nc.tensor.matmul(
                        ps,
                        xtb[n][:, kc, (sc % 4) * 128:(sc % 4 + 1) * 128],
                        wv_sb[:, kc, :],
                        start=(kc == 0), stop=(kc == 7),
                    )
                nc.vector.tensor_add(v_sb[:, sc, :], ps, bv_sb)

            def pa_group_closures(n):
                cl = []
                for m in range(2):
                    cl.append(lambda n=n, m=m: emit_qk_group(n, m, "q"))
                    cl.append(lambda n=n, m=m: emit_qk_group(n, m, "k"))
                for sc in range(4 * n, 4 * n + 4):
                    cl.append(lambda n=n, sc=sc: emit_v_group(n, sc))
                return cl

            # ---- phase B: attention per q tile, software-pipelined ----
            # Two heads are processed together at score-pair granularity so
            # the PE always has independent matmuls to run while the
            # exp (scalar) -> causal-mask select (gpsimd) chain of the
            # previous pair is still in flight.
            cn_of = {}

            def emit_scores(qi, h, g):
                mc = h // 2
                fo = (h % 2) * HD
                qsl = slice(qi * QT, (qi + 1) * QT)
                st = st_ps.tile([128, len(g), QT], F32, tag="st")
                for si, (ki, kind) in enumerate(g):
                    nc.tensor.matmul(
                        st[:, si, :],
                        kt_sb[fo:fo + HD, mc, ki * 128:(ki + 1) * 128],
                        qt_sb[fo:fo + HD, mc, qsl],
                        start=True, stop=True,
                    )
                ex = work.tile([128, len(g), QT], DT)
                nc.scalar.activation(
                    out=ex, in_=st,
                    func=mybir.ActivationFunctionType.Exp, scale=0.125,
                )
                kinds = [kind for _, kind in g]
                if any(k[0] == "stair" for k in kinds) and all(
                    k[0] in ("stair", "full") for k in kinds
                ):
                    # keep iff f - p - c >= 0 per half (c=-512: keep all)
                    cs = [(k[1] if k[0] == "stair" else -QT) for k in kinds]
                    ex2 = work.tile([128, len(g), QT], DT, tag="ex2")
                    pat = (
                        [[cs[0] - cs[1], 2], [1, QT]]
                        if len(g) == 2 else [[1, QT]]
                    )
                    nc.gpsimd.affine_select(
                        out=ex2, in_=ex,
                        pattern=pat,
                        compare_op=mybir.AluOpType.is_ge,
                        fill=0.0, base=-cs[0], channel_multiplier=-1,
                    )
                    ex = ex2
                else:
                    for si, k in enumerate(kinds):
                        if k[0] == "full":
                            continue
                        ex2 = work.tile([128, QT], DT, tag="exm")
                        if k[0] == "stair":
                            nc.gpsimd.affine_select(
                                out=ex2, in_=ex[:, si, :],
                                pattern=[[1, QT]],
                                compare_op=mybir.AluOpType.is_ge,
                                fill=0.0, base=-k[1], channel_multiplier=-1,
                            )
                        else:
                            nc.gpsimd.tensor_mul(
                                ex2, ex[:, si, :], mp_sb[:, k[1], :]
                            )
                        nc.gpsimd.tensor_copy(ex[:, si, :], ex2)
                return ex

            def emit_ctx(ctx, h, g, ex, start, stop):
                for si, (ki, kind) in enumerate(g):
                    nc.tensor.matmul(
                        ctx,
                        v_sb[:, ki, h * (HD + 1):(h + 1) * (HD + 1)],
                        ex[:, si, :],
                        start=(start and si == 0),
                        stop=(stop and si == len(g) - 1),
                    )

            def attn_core(qi, fillers, prev_norms):
                cn0 = ctxn.tile([128, QT], DT, tag="cn0")
                cn1 = ctxn.tile([128, QT], DT, tag="cn1")
                cn = [cn0, cn1]
                cn_of[qi] = cn
                chunks = blocks[qi]
                groups = [chunks[i:i + 2] for i in range(0, len(chunks), 2)]
                ng = len(groups)
                deferred = []
                units_left = [2 * (ng + 1)]

                def unit_done():
                    # one pipeline stage finished: flush the previous q-tile's
                    # deferred norm PE work first, then spread the filler
                    # groups (next phase-A block + previous output projection)
                    # evenly over this q-tile's stages
                    if prev_norms:
                        for fn in prev_norms:
                            fn()
                        prev_norms.clear()
                    units_left[0] -= 1
                    if fillers:
                        k = -(-len(fillers) // max(1, units_left[0]))
                        for _ in range(min(k, len(fillers))):
                            fillers.pop(0)()

                def norm_pe(h, ctx, dn_sb):
                    even = (h % 2 == 0)
                    mc = h // 2
                    bc = pe_ps.tile([HD, QT], F32, tag="pe")
                    nc.tensor.matmul(
                        bc, ones_r[HD:HD + 1, :], dn_sb[HD:HD + 1, :],
                        start=True, stop=True,
                    )
                    rc = norm.tile([HD, QT], F32, tag="rc")
                    nc.vector.reciprocal_approx_fast(out=rc, in_=bc)
                    if even:
                        nc.vector.tensor_mul(cn[mc][0:HD, :], ctx[0:HD, :], rc)
                    else:
                        tmp2 = norm.tile([HD, QT], DT, tag="tmp2")
                        nc.vector.tensor_mul(tmp2, ctx[0:HD, :], rc)
                        nc.sync.dma_start(out=cn[mc][HD:2 * HD, :], in_=tmp2)

                for hp, heads in enumerate(((2, 3), (0, 1))):
                    ctxs = {
                        h: ctx_ps.tile(
                            [HD + 1, QT], F32, tag="ctx", name=f"ctx{h}"
                        )
                        for h in heads
                    }
                    exs = {}
                    for g in range(ng):
                        for h in heads:
                            exs[(h, g)] = emit_scores(qi, h, groups[g])
                        if hp == 1 and g == 1 and deferred:
                            # previous head-pair's norm PE work, emitted here
                            # so its denominator copy has had time to land
                            for fn in deferred:
                                fn()
                            deferred = []
                        if g >= 1:
                            for h in heads:
                                emit_ctx(ctxs[h], h, groups[g - 1],
                                         exs.pop((h, g - 1)),
                                         start=(g == 1), stop=False)
                        unit_done()
                    for h in heads:
                        emit_ctx(ctxs[h], h, groups[ng - 1],
                                 exs.pop((h, ng - 1)),
                                 start=(ng == 1), stop=True)
                    unit_done()
                    for h in heads:
                        dn_sb = norm.tile([HD + 1, QT], F32R, tag="dn")
                        with nc.allow_low_precision(reason="f32r bcast operand"):
                            nc.vector.tensor_copy(
                                dn_sb[HD:HD + 1, :], ctxs[h][HD:HD + 1, :]
                            )
                        deferred.append(
                            (lambda h=h, c=ctxs[h], d=dn_sb: norm_pe(h, c, d))
                        )
                return deferred

            def emit_op_group(qi, qc):
                cn = cn_of[qi]
                so = stage.tile([128, 2, QT], DT)
                for ne in range(2):
                    ps = pe_ps.tile([128, QT], F32, tag="pe", name="psop")
                    for cc in (1, 0):
                        nc.tensor.matmul(
                            ps,
                            cn[cc][:, qc * 128:(qc + 1) * 128],
                            wo_sb[:, cc, ne * QT:(ne + 1) * QT],
                            start=(cc == 1), stop=(cc == 0),
                        )
                    nc.vector.tensor_copy(so[:, ne, :], ps)
                nc.sync.dma_start(
                    out=out_d[qi * QT + qc * 128: qi * QT + (qc + 1) * 128, :],
                    in_=so,
                )
                if qc == 3:
                    cn_of.pop(qi)

            # schedule: phase-A GEMM groups and output projections are fed as
            # "filler" PE work between attention pipeline stages, so the
            # scalar engine (exp) is never the serial bottleneck and the PE
            # never drains while softmax chains complete.
            ready_at = [max(ki for ki, _ in blocks[qi]) // 4 for qi in range(NQT)]
            order = sorted(range(NQT), key=lambda qi: (ready_at[qi], qi))
            emitted = 0            # phase-A blocks emitted (or queued as filler)
            pending_norms = []
            prev_qi = None
            for idx, qi in enumerate(order):
                while emitted <= ready_at[qi]:
                    for fn in pa_group_closures(emitted):
                        fn()
                    emitted += 1
                next_need = (
                    ready_at[order[idx + 1]] + 1 if idx + 1 < len(order) else emitted
                )
                fillers = []
                pa_fill = []
                while emitted < next_need:
                    pa_fill += pa_group_closures(emitted)
                    emitted += 1
                op_fill = (
                    [
                        (lambda q=prev_qi, c=qc: emit_op_group(q, c))
                        for qc in range(4)
                    ]
                    if prev_qi is not None else []
                )
                # interleave: two phase-A groups first (they only depend on
                # DMA-resident data), then alternate with outproj groups
                ia = ib = 0
                while ia < len(pa_fill) or ib < len(op_fill):
                    if ia < len(pa_fill):
                        fillers.append(pa_fill[ia])
                        ia += 1
                    if (ia >= 2 or ia >= len(pa_fill)) and ib < len(op_fill):
                        fillers.append(op_fill[ib])
                        ib += 1
                pending_norms = attn_core(qi, fillers, pending_norms)
                prev_qi = qi
            for fn in pending_norms:
                fn()
            for qc in range(4):
                emit_op_group(prev_qi, qc)

    nc.compile()
    return nc


def _block_structure(mask):
    """Classify [QT x KC] score blocks from the runtime mask (mask[q, k])."""
    allowed = ~np.isneginf(np.asarray(mask, dtype=np.float32))
    pats = []
    pat_idx = {}
    blocks = []
    for qi in range(NQT):
        row = []
        for ki in range(NKC):
            sub = allowed[qi * QT:(qi + 1) * QT, ki * KC:(ki + 1) * KC]
            if not sub.any():
                continue
            if sub.all():
                row.append((ki, FULL))
                continue
            # staircase: keep (p, f) iff f >= p + c, in transposed [k, q] view
            subT = sub.T  # [128 k, 512 q]
            c = None
            for cand in range(-QT + 1, KC + QT):
                ref = (np.arange(QT)[None, :] >= np.arange(KC)[:, None] + cand)
                if np.array_equal(subT, ref):
                    c = cand
                    break
            if c is not None:
                row.append((ki, ("stair", c)))
            else:
                pat = np.ascontiguousarray(
                    np.where(subT, 1.0, 0.0).astype(np.float32)
                )  # [128, 512] multiplicative mask
                key = pat.tobytes()
                if key not in pat_idx:
                    pat_idx[key] = len(pats)
                    pats.append(pat)
                row.append((ki, ("mask", pat_idx[key])))
        blocks.append(tuple(row))
    return tuple(blocks), pats


def kernel(x, mask, Wq, bq, Wk, bk, Wv, bv, Wo, bo):
    x = np.asarray(x, dtype=np.float32)
    blocks, pats = _block_structure(mask)
    n_pat = len(pats)
    key = (blocks, n_pat, USE_BF16)
    if key not in _cache:
        _cache[key] = _build(blocks, n_pat)
    nc = _cache[key]

    xt = [np.ascontiguousarray(x[b].T).astype(NPDT) for b in range(B)]
    in_maps = []
    for c in range(NCORES):
        b, hg = c // HPC, c % HPC
        hs = slice(hg * DPC, (hg + 1) * DPC)
        wv_aug = np.zeros((D, VW), dtype=np.float32)
        bv_aug = np.zeros(VW, dtype=np.float32)
        for j in range(HPC):
            base = j * (HD + 1)
            rows = slice(hg * DPC + j * HD, hg * DPC + (j + 1) * HD)
            wv_aug[:, base:base + HD] = np.asarray(Wv)[rows, :].T
            bv_aug[base:base + HD] = np.asarray(bv)[rows]
            bv_aug[base + HD] = 1.0
        im = {
            "xt": xt[b],
            "wq": np.ascontiguousarray(np.asarray(Wq)[hs, :].T).astype(NPDT),
            "wk": np.ascontiguousarray(np.asarray(Wk)[hs, :].T).astype(NPDT),
            "wv": wv_aug.astype(NPDT),
            "wo": np.ascontiguousarray(np.asarray(Wo)[:, hs].T).astype(NPDT),
            "bq": np.ascontiguousarray(np.asarray(bq)[hs].reshape(2, 128).T),
            "bk": np.ascontiguousarray(np.asarray(bk)[hs].reshape(2, 128).T),
            "bv": bv_aug,
        }
        if n_pat:
            im["mp"] = np.concatenate(pats, axis=1).astype(NPDT)
        in_maps.append(im)

    res = run_bass_kernel_spmd(nc, in_maps, core_ids=list(range(NCORES))).results
    out = np.empty((B, S, D), dtype=np.float32)
    for b in range(B):
        acc = res[b * HPC]["out"].astype(np.float32)
        for g in range(1, HPC):
            acc = acc + res[b * HPC + g]["out"].astype(np.float32)
        out[b] = acc + np.asarray(bo, dtype=np.float32)[None, :]
    return out


# revision 31
# speedup vs baseline: 1.2993x; 1.0047x over previous
"""Causal self-attention on 8 trn2 NeuronCores.

Sharding: core c handles batch b = c//4 and heads 4*(c%4) .. 4*(c%4)+3
(data parallel on B, tensor parallel on the 16 heads). Each core computes
its 4 heads' attention plus the corresponding slice of the output
projection; the host sums the 4 partial projections per batch and adds bo.

On-chip layout is feature-major ("transposed"): qT/kT are [head_dim, seq],
scores are computed as sT[k, q] so the attention@v matmul needs no
transposes. The softmax denominator comes from an extra all-ones column
appended to Wv (so ctx PSUM row 64 accumulates sum_k exp). Normalization
broadcasts 1/denom across partitions via a tiny K=1 matmul.

Matmuls run in bfloat16 (hw-measured 2x faster than fp32/f32r for both
MATMUL and LDWEIGHTS on trn2). Causal masking is done with gpsimd
affine_select on the exp'd scores (keep iff q >= k, i.e. f - p - c >= 0
in the transposed block layout) instead of PE mask-add matmuls. Score
blocks are exp'd two at a time from a 2-bank PSUM tile to halve the
scalar-engine instruction count.
"""

import os
import sys

sys.path.insert(0, "/opt/trn_rl_repo")

import numpy as np
import ml_dtypes

import concourse.bass as bass
import concourse.tile as tile
from concourse import bacc, mybir
from concourse.bass_utils import run_bass_kernel_spmd

B, S, D, H = 2, 2048, 1024, 16
HD = D // H            # 64
NCORES = 8
HPC = 4                # heads per core
DPC = HPC * HD         # 256 feature dims per core
QT = 512               # q tile (free dim of score matmuls)
KC = 128               # k chunk (partition dim of transposed scores)
NQT = S // QT          # 4
NKC = S // KC          # 16
VW = HPC * (HD + 1)    # 260: v with ones column per head

F32 = mybir.dt.float32
F32R = mybir.dt.float32r
USE_BF16 = os.environ.get("KBF16", "1") == "1"
DT = mybir.dt.bfloat16 if USE_BF16 else F32R
NPDT = ml_dtypes.bfloat16 if USE_BF16 else np.float32

_cache = {}

# chunk kinds in the block structure
FULL = ("full",)


def _dram_ap(t, offset, dims):
    """dims: list of (elem_stride, n). Builds a raw AP on a dram tensor."""
    return bass.AP(tensor=t.tensor, offset=offset, ap=[list(d) for d in dims])


def _build(blocks, n_pat):
    """blocks: per q-tile, tuple of (ki, kind) chunks; kind is FULL,
    ('stair', c) with keep iff f - p - c >= 0, or ('mask', pat_idx)."""
    nc = bacc.Bacc(
        "TRN2",
        target_bir_lowering=False,
        debug=False,
        enable_asserts=False,
        num_devices=NCORES,
    )

    xt_d = nc.dram_tensor("xt", [D, S], DT, kind="ExternalInput").ap()
    wq_d = nc.dram_tensor("wq", [D, DPC], DT, kind="ExternalInput").ap()
    wk_d = nc.dram_tensor("wk", [D, DPC], DT, kind="ExternalInput").ap()
    wv_d = nc.dram_tensor("wv", [D, VW], DT, kind="ExternalInput").ap()
    wo_d = nc.dram_tensor("wo", [DPC, D], DT, kind="ExternalInput").ap()
    bq_d = nc.dram_tensor("bq", [128, 2], F32, kind="ExternalInput").ap()
    bk_d = nc.dram_tensor("bk", [128, 2], F32, kind="ExternalInput").ap()
    bv_d = nc.dram_tensor("bv", [VW], F32, kind="ExternalInput").ap()
    if n_pat:
        mp_d = nc.dram_tensor("mp", [128, n_pat * QT], DT, kind="ExternalInput").ap()
    out_d = nc.dram_tensor("out", [S, D], DT, kind="ExternalOutput").ap()

    with tile.TileContext(nc) as tc:
        with (
            tc.tile_pool(name="consts", bufs=1) as consts,
            tc.tile_pool(name="pe_ps", bufs=2, space="PSUM") as pe_ps,
            tc.tile_pool(name="st_ps", bufs=2, space="PSUM") as st_ps,
            tc.tile_pool(name="ctx_ps", bufs=2, space="PSUM") as ctx_ps,
            tc.tile_pool(name="work", bufs=8) as work,
            tc.tile_pool(name="norm", bufs=4) as norm,
            tc.tile_pool(name="ctxn", bufs=4) as ctxn,
            tc.tile_pool(name="stage", bufs=3) as stage,
        ):
            # ---- resident loads: one DMA descriptor per tensor, ordered so
            # the first q matmuls can start as early as possible ----
            wq_sb = consts.tile([128, 8, DPC], DT)
            xtb = []
            for nb in range(NQT):
                xtb_t = consts.tile([128, 8, QT], DT, tag=f"xtb{nb}")
                xtb.append(xtb_t)
            # first-needed tensors split in halves so compute starts sooner
            for ha in range(2):
                nc.sync.dma_start(
                    out=wq_sb[:, ha * 4:(ha + 1) * 4, :],
                    in_=_dram_ap(
                        wq_d, ha * 4 * 128 * DPC,
                        [(DPC, 128), (128 * DPC, 4), (1, DPC)],
                    ),
                )
                nc.sync.dma_start(
                    out=xtb[0][:, ha * 4:(ha + 1) * 4, :],
                    in_=_dram_ap(
                        xt_d, ha * 4 * 128 * S,
                        [(S, 128), (128 * S, 4), (1, QT)],
                    ),
                )
            wk_sb = consts.tile([128, 8, DPC], DT)
            wv_sb = consts.tile([128, 8, VW], DT)
            nc.sync.dma_start(
                out=wk_sb,
                in_=_dram_ap(wk_d, 0, [(DPC, 128), (128 * DPC, 8), (1, DPC)]),
            )
            nc.sync.dma_start(
                out=wv_sb,
                in_=_dram_ap(wv_d, 0, [(VW, 128), (128 * VW, 8), (1, VW)]),
            )
            bq_sb = consts.tile([128, 2], F32)
            bk_sb = consts.tile([128, 2], F32)
            bv_sb = consts.tile([128, VW], F32)
            nc.sync.dma_start(out=bq_sb, in_=bq_d)
            nc.sync.dma_start(out=bk_sb, in_=bk_d)
            nc.sync.dma_start(out=bv_sb, in_=_dram_ap(bv_d, 0, [(0, 128), (1, VW)]))
            for nb in range(1, NQT):
                nc.sync.dma_start(
                    out=xtb[nb],
                    in_=_dram_ap(xt_d, nb * QT, [(S, 128), (128 * S, 8), (1, QT)]),
                )
            wo_sb = consts.tile([128, 2, D], DT)
            nc.sync.dma_start(
                out=wo_sb, in_=_dram_ap(wo_d, 0, [(D, 128), (128 * D, 2), (1, D)])
            )
            if n_pat:
                mp_sb = consts.tile([128, n_pat, QT], DT)
                nc.sync.dma_start(
                    out=mp_sb,
                    in_=_dram_ap(
                        mp_d, 0, [(n_pat * QT, 128), (QT, n_pat), (1, QT)]
                    ),
                )

            ones_f = consts.tile([65, HD], F32)
            nc.vector.memset(ones_f, 1.0)
            ones_r = consts.tile([65, HD], F32R)
            nc.vector.tensor_copy(ones_r, ones_f)

            # ---- phase A: qT/kT = W @ xT, v = x @ Wv_aug (feature-major q/k) ----
            qt_sb = consts.tile([128, 2, S], DT)
            kt_sb = consts.tile([128, 2, S], DT)
            v_sb = consts.tile([128, NKC, VW], DT)

            def emit_qk_group(n, m, which):
                w_sb, b_sb, t_sb = (
                    (wq_sb, bq_sb, qt_sb) if which == "q" else (wk_sb, bk_sb, kt_sb)
                )
                ps = pe_ps.tile([128, QT], F32, tag="pe", name="psqk")
                for kc in range(8):
                    nc.tensor.matmul(
                        ps,
                        w_sb[:, kc, m * 128:(m + 1) * 128],
                        xtb[n][:, kc, :],
                        start=(kc == 0), stop=(kc == 7),
                    )
                nc.vector.tensor_scalar_add(
                    t_sb[:, m, n * QT:(n + 1) * QT], ps, b_sb[:, m:m + 1]
                )

            def emit_v_group(n, sc):
                ps = pe_ps.tile([128, VW], F32, tag="pe", name="psv")
                for kc in range(8):
                    nc.tensor.matmul(
                        ps,
                        xtb[n][:, kc, (sc % 4) * 128:(sc % 4 + 1) * 128],
                        wv_sb[:, kc, :],
                        start=(kc == 0), stop=(kc == 7),
                    )
                nc.vector.tensor_add(v_sb[:, sc, :], ps, bv_sb)

            def pa_group_closures(n):
                cl = []
                for m in range(2):
                    cl.append(lambda n=n, m=m: emit_qk_group(n, m, "q"))
                    cl.append(lambda n=n, m=m: emit_qk_group(n, m, "k"))
                for sc in range(4 * n, 4 * n + 4):
                    cl.append(lambda n=n, sc=sc: emit_v_group(n, sc))
                return cl

            # ---- phase B: attention per q tile, software-pipelined ----
            # Two heads are processed together at score-pair granularity so
            # the PE always has independent matmuls to run while the
            # exp (scalar) -> causal-mask select (gpsimd) chain of the
            # previous pair is still in flight.
            cn_of = {}

            def emit_scores(qi, h, g):
                mc = h // 2
                fo = (h % 2) * HD
                qsl = slice(qi * QT, (qi + 1) * QT)
                st = st_ps.tile([128, len(g), QT], F32, tag="st")
                for si, (ki, kind) in enumerate(g):
                    nc.tensor.matmul(
                        st[:, si, :],
                        kt_sb[fo:fo + HD, mc, ki * 128:(ki + 1) * 128],
                        qt_sb[fo:fo + HD, mc, qsl],
                        start=True, stop=True,
                    )
                ex = work.tile([128, len(g), QT], DT)
                nc.scalar.activation(
                    out=ex, in_=st,
                    func=mybir.ActivationFunctionType.Exp, scale=0.125,
                )
                kinds = [kind for _, kind in g]
                if any(k[0] == "stair" for k in kinds) and all(
                    k[0] in ("stair", "full") for k in kinds
                ):
                    # keep iff f - p - c >= 0 per half (c=-512: keep all)
                    cs = [(k[1] if k[0] == "stair" else -QT) for k in kinds]
                    ex2 = work.tile([128, len(g), QT], DT, tag="ex2")
                    pat = (
                        [[cs[0] - cs[1], 2], [1, QT]]
                        if len(g) == 2 else [[1, QT]]
                    )
                    nc.gpsimd.affine_select(
                        out=ex2, in_=ex,
                        pattern=pat,
                        compare_op=mybir.AluOpType.is_ge,
                        fill=0.0, base=-cs[0], channel_multiplier=-1,
                    )
                    ex = ex2
                else:
                    for si, k in enumerate(kinds):
                        if k[0] == "full":
                            continue
                        ex2 = work.tile([128, QT], DT, tag="exm")
                        if k[0] == "stair":
                            nc.gpsimd.affine_select(
                                out=ex2, in_=ex[:, si, :],
                                pattern=[[1, QT]],
                                compare_op=mybir.AluOpType.is_ge,
                                fill=0.0, base=-k[1], channel_multiplier=-1,
                            )
                        else:
                            nc.gpsimd.tensor_mul(
                                ex2, ex[:, si, :], mp_sb[:, k[1], :]
                            )
                        nc.gpsimd.tensor_copy(ex[:, si, :], ex2)
                return ex

            def emit_ctx(ctx, h, g, ex, start, stop):
                for si, (ki, kind) in enumerate(g):
                    nc.tensor.matmul(
                        ctx[0:HD + 1, :],
                        v_sb[:, ki, h * (HD + 1):(h + 1) * (HD + 1)],
                        ex[:, si, :],
                        start=(start and si == 0),
                        stop=(stop and si == len(g) - 1),
                    )

            def attn_core(qi, fillers, prev_norms):
                cn0 = ctxn.tile([128, QT], DT, tag="cn0")
                cn1 = ctxn.tile([128, QT], DT, tag="cn1")
                cn = [cn0, cn1]
                cn_of[qi] = cn
                chunks = blocks[qi]
                groups = [chunks[i:i + 2] for i in range(0, len(chunks), 2)]
                ng = len(groups)
                deferred = []
                units_left = [2 * (ng + 1)]

                def unit_done():
                    # one pipeline stage finished: flush the previous q-tile's
                    # deferred norm PE work first, then spread the filler
                    # groups (next phase-A block + previous output projection)
                    # evenly over this q-tile's stages
                    if prev_norms:
                        for fn in prev_norms:
                            fn()
                        prev_norms.clear()
                    units_left[0] -= 1
                    if fillers:
                        k = -(-len(fillers) // max(1, units_left[0]))
                        for _ in range(min(k, len(fillers))):
                            fillers.pop(0)()

                def norm_pe(h, ctx, dn_sb):
                    even = (h % 2 == 0)
                    mc = h // 2
                    bc = pe_ps.tile([HD, QT], F32, tag="pe", name="bc")
                    nc.tensor.matmul(
                        bc, ones_r[HD:HD + 1, :], dn_sb[HD:HD + 1, :],
                        start=True, stop=True,
                    )
                    rc = norm.tile([HD, QT], F32, tag="rc")
                    nc.vector.reciprocal_approx_fast(out=rc, in_=bc)
                    # engines address operands at independent 32-aligned base
                    # partitions, so the odd head writes partitions 64-127 of
                    # cn directly while reading ctx/rc at base 0
                    dst = cn[mc][0:HD, :] if even else cn[mc][HD:2 * HD, :]
                    nc.vector.tensor_mul(dst, ctx[0:HD, :], rc)

                for hp, heads in enumerate(((2, 3), (0, 1))):
                    ctxs = {
                        h: ctx_ps.tile(
                            [128, QT], F32, tag="ctx", name=f"ctx{h}"
                        )
                        for h in heads
                    }
                    exs = {}
                    for g in range(ng):
                        for h in heads:
                            exs[(h, g)] = emit_scores(qi, h, groups[g])
                        if hp == 1 and g == 1 and deferred:
                            # previous head-pair's norm PE work, emitted here
                            # so its denominator copy has had time to land
                            for fn in deferred:
                                fn()
                            deferred = []
                        if g >= 1:
                            for h in heads:
                                emit_ctx(ctxs[h], h, groups[g - 1],
                                         exs.pop((h, g - 1)),
                                         start=(g == 1), stop=False)
                        unit_done()
                    for h in heads:
                        emit_ctx(ctxs[h], h, groups[ng - 1],
                                 exs.pop((h, ng - 1)),
                                 start=(ng == 1), stop=True)
                    unit_done()
                    for h in heads:
                        dn_sb = norm.tile([HD + 1, QT], F32R, tag="dn")
                        with nc.allow_low_precision(reason="f32r bcast operand"):
                            nc.vector.tensor_copy(
                                dn_sb[HD:HD + 1, :], ctxs[h][HD:HD + 1, :]
                            )
                        deferred.append(
                            (lambda h=h, c=ctxs[h], d=dn_sb: norm_pe(h, c, d))
                        )
                return deferred

            def emit_op_group(qi, qc, final=False):
                cn = cn_of[qi]
                so = stage.tile([128, 2, QT], DT)
                if final:
                    # attention is done: the score-PSUM banks are free, use
                    # them as double-width outproj accumulators so the PE
                    # never waits on single-bank evacuation at the drain
                    ps2 = st_ps.tile([128, 2, QT], F32, tag="st", name="psfin")
                    for ne in range(2):
                        for cc in (1, 0):
                            nc.tensor.matmul(
                                ps2[:, ne, :],
                                cn[cc][:, qc * 128:(qc + 1) * 128],
                                wo_sb[:, cc, ne * QT:(ne + 1) * QT],
                                start=(cc == 1), stop=(cc == 0),
                            )
                    nc.vector.tensor_copy(so, ps2)
                else:
                    for ne in range(2):
                        ps = pe_ps.tile([128, QT], F32, tag="pe", name="psop")
                        for cc in (1, 0):
                            nc.tensor.matmul(
                                ps,
                                cn[cc][:, qc * 128:(qc + 1) * 128],
                                wo_sb[:, cc, ne * QT:(ne + 1) * QT],
                                start=(cc == 1), stop=(cc == 0),
                            )
                        nc.vector.tensor_copy(so[:, ne, :], ps)
                nc.sync.dma_start(
                    out=out_d[qi * QT + qc * 128: qi * QT + (qc + 1) * 128, :],
                    in_=so,
                )
                if qc == 3:
                    cn_of.pop(qi)

            # schedule: phase-A GEMM groups and output projections are fed as
            # "filler" PE work between attention pipeline stages, so the
            # scalar engine (exp) is never the serial bottleneck and the PE
            # never drains while softmax chains complete.
            ready_at = [max(ki for ki, _ in blocks[qi]) // 4 for qi in range(NQT)]
            order = sorted(range(NQT), key=lambda qi: (ready_at[qi], qi))
            emitted = 0            # phase-A blocks emitted (or queued as filler)
            pending_norms = []
            prev_qi = None
            for idx, qi in enumerate(order):
                while emitted <= ready_at[qi]:
                    for fn in pa_group_closures(emitted):
                        fn()
                    emitted += 1
                next_need = (
                    ready_at[order[idx + 1]] + 1 if idx + 1 < len(order) else emitted
                )
                fillers = []
                pa_fill = []
                while emitted < next_need:
                    pa_fill += pa_group_closures(emitted)
                    emitted += 1
                op_fill = (
                    [
                        (lambda q=prev_qi, c=qc: emit_op_group(q, c))
                        for qc in range(4)
                    ]
                    if prev_qi is not None else []
                )
                # interleave: two phase-A groups first (they only depend on
                # DMA-resident data), then alternate with outproj groups
                ia = ib = 0
                while ia < len(pa_fill) or ib < len(op_fill):
                    if ia < len(pa_fill):
                        fillers.append(pa_fill[ia])
                        ia += 1
                    if (ia >= 2 or ia >= len(pa_fill)) and ib < len(op_fill):
                        fillers.append(op_fill[ib])
                        ib += 1
                pending_norms = attn_core(qi, fillers, pending_norms)
                prev_qi = qi
            for fn in pending_norms:
                fn()
            for qc in range(4):
                emit_op_group(prev_qi, qc, final=True)

    nc.compile()
    return nc


def _block_structure(mask):
    """Classify [QT x KC] score blocks from the runtime mask (mask[q, k])."""
    allowed = ~np.isneginf(np.asarray(mask, dtype=np.float32))
    pats = []
    pat_idx = {}
    blocks = []
    for qi in range(NQT):
        row = []
        for ki in range(NKC):
            sub = allowed[qi * QT:(qi + 1) * QT, ki * KC:(ki + 1) * KC]
            if not sub.any():
                continue
            if sub.all():
                row.append((ki, FULL))
                continue
            # staircase: keep (p, f) iff f >= p + c, in transposed [k, q] view
            subT = sub.T  # [128 k, 512 q]
            c = None
            for cand in range(-QT + 1, KC + QT):
                ref = (np.arange(QT)[None, :] >= np.arange(KC)[:, None] + cand)
                if np.array_equal(subT, ref):
                    c = cand
                    break
            if c is not None:
                row.append((ki, ("stair", c)))
            else:
                pat = np.ascontiguousarray(
                    np.where(subT, 1.0, 0.0).astype(np.float32)
                )  # [128, 512] multiplicative mask
                key = pat.tobytes()
                if key not in pat_idx:
                    pat_idx[key] = len(pats)
                    pats.append(pat)
                row.append((ki, ("mask", pat_idx[key])))
        blocks.append(tuple(row))
    return tuple(blocks), pats


def kernel(x, mask, Wq, bq, Wk, bk, Wv, bv, Wo, bo):
    x = np.asarray(x, dtype=np.float32)
    blocks, pats = _block_structure(mask)
    n_pat = len(pats)
    key = (blocks, n_pat, USE_BF16)
    if key not in _cache:
        _cache[key] = _build(blocks, n_pat)
    nc = _cache[key]

    xt = [np.ascontiguousarray(x[b].T).astype(NPDT) for b in range(B)]
    in_maps = []
    for c in range(NCORES):
        b, hg = c // HPC, c % HPC
        hs = slice(hg * DPC, (hg + 1) * DPC)
        wv_aug = np.zeros((D, VW), dtype=np.float32)
        bv_aug = np.zeros(VW, dtype=np.float32)
        for j in range(HPC):
            base = j * (HD + 1)
            rows = slice(hg * DPC + j * HD, hg * DPC + (j + 1) * HD)
            wv_aug[:, base:base + HD] = np.asarray(Wv)[rows, :].T
            bv_aug[base:base + HD] = np.asarray(bv)[rows]
            bv_aug[base + HD] = 1.0
        im = {
            "xt": xt[b],
            "wq": np.ascontiguousarray(np.asarray(Wq)[hs, :].T).astype(NPDT),
            "wk": np.ascontiguousarray(np.asarray(Wk)[hs, :].T).astype(NPDT),
            "wv": wv_aug.astype(NPDT),
            "wo": np.ascontiguousarray(np.asarray(Wo)[:, hs].T).astype(NPDT),
            "bq": np.ascontiguousarray(np.asarray(bq)[hs].reshape(2, 128).T),
            "bk": np.ascontiguousarray(np.asarray(bk)[hs].reshape(2, 128).T),
            "bv": bv_aug,
        }
        if n_pat:
            im["mp"] = np.concatenate(pats, axis=1).astype(NPDT)
        in_maps.append(im)

    res = run_bass_kernel_spmd(nc, in_maps, core_ids=list(range(NCORES))).results
    out = np.empty((B, S, D), dtype=np.float32)
    for b in range(B):
        acc = res[b * HPC]["out"].astype(np.float32)
        for g in range(1, HPC):
            acc = acc + res[b * HPC + g]["out"].astype(np.float32)
        out[b] = acc + np.asarray(bo, dtype=np.float32)[None, :]
    return out
